# revision 7
# baseline (speedup 1.0000x reference)
"""Distributed GQA attention block (dense_transformer) for 8 TRN2 NeuronCores.

Sharding: Megatron-style head sharding for QKV+attention (each core owns 4 Q
heads / 1 KV head), Ulysses-style AllToAll to switch to sequence sharding for
the output projection (each core owns 256 rows per batch).

Layouts (per core, SPMD identical graph):
  - All activations kept transposed: QT/KT [head_dim, rows] so scores come out
    as S^T [k, q] and softmax reduces over the partition axis via matmul with a
    fused ones-column in V (denominator for free).
  - RoPE handled by permuting head dims (evens|odds) in the weights on the
    host, so rotation pairs are partition halves: out = qn*cos + swap(qn)*sin±.
  - RMSNorm partition-reduction via indicator matmul; rsqrt via ACT ln/exp
    (same ACT table set as softmax exp -> zero table switches).
Compute in bf16 on the TensorEngine (f32 accumulation), f32 softmax.
"""
import sys

if '/opt/trn_rl_repo' not in sys.path:
    sys.path.insert(0, '/opt/trn_rl_repo')

import numpy as np
import ml_dtypes

N_CORES = 8
B, S, D = 2, 2048, 2048
DH = 64
HLOC = 4            # Q heads per core
ROWS = B * S        # 4096
NKT = D // 128      # 16 contraction tiles
RC = 512            # row chunk
NCHUNK = ROWS // RC # 8
EPS = 1e-6

_cache = {}


def _build():
    import concourse.mybir as mybir
    import concourse.tile as tile
    from concourse import bacc
    from concourse.bass import ts, ds

    f32 = mybir.dt.float32
    bf = mybir.dt.bfloat16
    AF = mybir.ActivationFunctionType
    MUL = mybir.AluOpType.mult

    nc = bacc.Bacc()
    x_sb = nc.declare_dram_parameter("x_sb", [128, NKT * ROWS], bf, isOutput=False)
    wq_sb = nc.declare_dram_parameter("wq_sb", [128, NKT * 2 * 128], bf, isOutput=False)
    wk_sb = nc.declare_dram_parameter("wk_sb", [128, NKT * 128], bf, isOutput=False)
    wv_sb = nc.declare_dram_parameter("wv_sb", [128, NKT * 64], bf, isOutput=False)
    wo_sb = nc.declare_dram_parameter("wo_sb", [128, 16 * 16 * 128], bf, isOutput=False)
    cos_sb = nc.declare_dram_parameter("cos_sb", [128, ROWS], f32, isOutput=False)
    sin_sb = nc.declare_dram_parameter("sin_sb", [128, ROWS], f32, isOutput=False)
    msk_sb = nc.declare_dram_parameter("msk_sb", [128, 896], bf, isOutput=False)
    ind_sb = nc.declare_dram_parameter("ind_sb", [128, 2], bf, isOutput=False)
    idn_sb = nc.declare_dram_parameter("idn_sb", [64, 64], bf, isOutput=False)
    sc_sb = nc.declare_dram_parameter("sc_sb", [64, 2], f32, isOutput=False)
    out_ext = nc.declare_dram_parameter("out", [D, 512], f32, isOutput=True)

    with tile.TileContext(nc) as tc:
        with (
            tc.tile_pool(name="cp", bufs=1) as cp,
            tc.tile_pool(name="xp", bufs=2) as xp,
            tc.tile_pool(name="wp", bufs=3) as wp,
            tc.tile_pool(name="sp", bufs=2) as sp,
            tc.tile_pool(name="ep", bufs=4) as ep,
            tc.tile_pool(name="dram", bufs=1, space="DRAM") as dram,
            tc.tile_pool(name="pproj", bufs=2, space="PSUM") as pproj,
            tc.tile_pool(name="pscore", bufs=2, space="PSUM") as pscore,
            tc.tile_pool(name="po", bufs=2, space="PSUM") as po,
            tc.tile_pool(name="psm", bufs=2, space="PSUM") as psm,
        ):
            # ---- persistent constants ----
            wq = cp.tile([128, NKT * 2 * 128], bf)
            for i in range(4):
                nc.sync.dma_start(wq[:, ds(i * 1024, 1024)], wq_sb[:, ds(i * 1024, 1024)])
            wkt = cp.tile([128, NKT * 128], bf)
            nc.sync.dma_start(wkt[:], wk_sb[:])
            wvt = cp.tile([128, NKT * 64], bf)
            nc.sync.dma_start(wvt[:], wv_sb[:])
            msk = cp.tile([128, 896], bf)
            nc.sync.dma_start(msk[:], msk_sb[:])
            ind = cp.tile([128, 2], bf)
            nc.sync.dma_start(ind[:], ind_sb[:])
            idn = cp.tile([64, 64], bf)
            nc.sync.dma_start(idn[:], idn_sb[:])
            sc2 = cp.tile([64, 2], f32)
            nc.sync.dma_start(sc2[:], sc_sb[:])
            epsc = cp.tile([1, 1], f32)
            nc.gpsimd.memset(epsc[:], EPS)
            ones64 = cp.tile([1, 64], bf)
            nc.gpsimd.memset(ones64[:], 1.0)

            QTn = [cp.tile([128, ROWS], bf, name=f"qtn{i}") for i in range(2)]
            KTd = [cp.tile([128, S], bf, name=f"ktd{b}") for b in range(B)]
            Vb1 = [cp.tile([128, 16 * 65], bf, name=f"vb{b}") for b in range(B)]
            attb = cp.tile([128, 16 * 256], bf)

            a2a_in = [dram.tile([2048, 256], f32, name=f"a2ain{b}") for b in range(B)]
            a2a_out = [dram.tile([2048, 256], f32, name=f"a2aout{b}") for b in range(B)]

            # ---- norm + rope on a projection psum tile ----
            def norm_rope(ps, dst_ap, cosc, sinc, sc_col, dup):
                sq = sp.tile([128, RC], bf, tag="sq")
                nc.scalar.activation(sq[:], ps[:], AF.Square)
                rbcs = []
                for h in range(1 if dup else 2):
                    ssp = psm.tile([1, RC], f32, tag="small", name=f"ss{h}")
                    nc.tensor.matmul(ssp[:], ind[:, h:h + 1], sq[:], start=True, stop=True)
                    lg = sp.tile([1, RC], f32, tag="lg", bufs=4)
                    nc.scalar.activation(lg[:], ssp[:], AF.Ln, scale=1.0 / 64, bias=epsc[:])
                    rstd = sp.tile([1, RC], f32, tag="rstd", bufs=4)
                    nc.scalar.activation(rstd[:], lg[:], AF.Exp, scale=-0.5)
                    rbc = sp.tile([64, RC], f32, tag="rbc", bufs=4, name=f"rbc{h}")
                    nc.gpsimd.partition_broadcast(rbc[:], rstd[:])
                    rbcs.append(rbc)
                if dup:
                    rbcs.append(rbcs[0])
                qn = sp.tile([128, RC], f32, tag="qn")
                nc.vector.scalar_tensor_tensor(
                    qn[0:64, :], ps[0:64, :], sc2[:, sc_col:sc_col + 1], rbcs[0][:], MUL, MUL)
                nc.vector.scalar_tensor_tensor(
                    qn[64:128, :], ps[64:128, :], sc2[:, sc_col:sc_col + 1], rbcs[1][:], MUL, MUL)
                swp = sp.tile([128, RC], f32, tag="swp")
                for g in range(4):
                    h = g // 2
                    nc.vector.scalar_tensor_tensor(
                        swp[ts(g, 32), :], ps[ts(g ^ 1, 32), :],
                        sc2[ts((g ^ 1) % 2, 32), sc_col:sc_col + 1],
                        rbcs[h][ts((g ^ 1) % 2, 32), :], MUL, MUL)
                nc.vector.tensor_mul(qn[:], qn[:], cosc[:])
                nc.vector.tensor_mul(swp[:], swp[:], sinc[:])
                nc.vector.tensor_add(dst_ap, qn[:], swp[:])

            # ---- one row-chunk of projections ----
            def proj_chunk(r):
                b, sl = r // 4, r % 4
                xt = xp.tile([128, NKT, RC], bf, tag="xt")
                nc.sync.dma_start(
                    xt[:], x_sb[:].rearrange("p (k q) -> p k q", k=NKT)[:, :, ds(r * RC, RC)])
                cosc = sp.tile([128, RC], f32, tag="cos")
                nc.sync.dma_start(cosc[:], cos_sb[:, ds(r * RC, RC)])
                sinc = sp.tile([128, RC], f32, tag="sin")
                nc.sync.dma_start(sinc[:], sin_sb[:, ds(r * RC, RC)])
                for hp in range(2):
                    psq = pproj.tile([128, RC], f32, tag="proj", name=f"psq{hp}")
                    for k in range(NKT):
                        nc.tensor.matmul(psq[:], wq[:, ds((k * 2 + hp) * 128, 128)],
                                         xt[:, k, :], start=(k == 0), stop=(k == NKT - 1))
                    norm_rope(psq, QTn[hp][:, ds(r * RC, RC)], cosc, sinc, 0, False)
                psk = pproj.tile([128, RC], f32, tag="proj")
                for k in range(NKT):
                    nc.tensor.matmul(psk[:], wkt[:, ts(k, 128)], xt[:, k, :],
                                     start=(k == 0), stop=(k == NKT - 1))
                norm_rope(psk, KTd[b][:, ds(sl * RC, RC)], cosc, sinc, 1, True)
                psv = pproj.tile([64, RC], f32, tag="proj")
                for k in range(NKT):
                    nc.tensor.matmul(psv[:], wvt[:, ts(k, 64)], xt[:, k, :],
                                     start=(k == 0), stop=(k == NKT - 1))
                vtmp = sp.tile([64, RC], bf, tag="vtmp")
                nc.vector.tensor_copy(vtmp[:], psv[:])
                for t4 in range(4):
                    tp = psm.tile([128, 64], bf, tag="small", name="tp")
                    nc.tensor.transpose(tp[:], vtmp[:, ts(t4, 128)], idn[:])
                    gt = sl * 4 + t4
                    nc.vector.tensor_copy(Vb1[b][:, ds(gt * 65, 64)], tp[:])
                    nc.vector.memset(Vb1[b][:, ds(gt * 65 + 64, 1)], 1.0)

            # ---- one attention block: batch b, head-pair hp, q-slice qs ----
            def attn_block(b, hp, qs):
                psO = [po.tile([65, RC], f32, tag="o", name=f"psO{t}") for t in range(2)]
                jmax = qs * 4 + 3
                for j in range(jmax + 1):
                    dj = j - qs * 4
                    p = dj * 128 if dj >= 0 else 0
                    N = RC - p
                    qb = b * S + qs * RC + p
                    psS = [pscore.tile([128, RC], f32, tag="s", name=f"psS{t}")
                           for t in range(2)]
                    nc.tensor.matmul(psS[0][:, 0:N], KTd[b][0:64, ts(j, 128)],
                                     QTn[hp][0:64, ds(qb, N)], start=True, stop=True,
                                     tile_position=(0, 0))
                    nc.tensor.matmul(psS[1][:, 0:N], KTd[b][64:128, ts(j, 128)],
                                     QTn[hp][64:128, ds(qb, N)], start=True, stop=True,
                                     tile_position=(64, 0))
                    for t in range(2):
                        E = ep.tile([128, RC], bf, tag="E")
                        if p:
                            nc.vector.memset(E[:, 0:p], 0.0)
                        nc.scalar.activation(E[:, p:RC], psS[t][:, 0:N], AF.Exp, scale=0.125)
                        if dj >= 0:
                            nc.vector.tensor_mul(E[:, ds(p, 128)], E[:, ds(p, 128)],
                                                 msk[:, ds(384, 128)])
                        nc.tensor.matmul(psO[t][:], Vb1[b][:, ds(j * 65, 65)], E[:],
                                         start=(j == 0), stop=(j == jmax))
                for t in range(2):
                    hl = 2 * hp + t
                    recip = sp.tile([1, RC], bf, tag="recip")
                    with nc.allow_low_precision(reason="softmax denom recip feeds bf16 bcast matmul"):
                        nc.vector.reciprocal(recip[:], psO[t][64:65, :])
                    bcp = psm.tile([64, RC], f32, tag="small", name="bcp")
                    nc.tensor.matmul(bcp[:], ones64[:], recip[:], start=True, stop=True)
                    rbco = sp.tile([64, RC], f32, tag="rbco")
                    nc.vector.tensor_copy(rbco[:], bcp[:])
                    on = sp.tile([64, RC], f32, tag="on")
                    nc.vector.tensor_mul(on[:], psO[t][0:64, :], rbco[:])
                    nc.sync.dma_start(
                        a2a_in[b][ds(256 * (2 * qs) + hl * 64, 64), :], on[:, 0:256])
                    nc.sync.dma_start(
                        a2a_in[b][ds(256 * (2 * qs + 1) + hl * 64, 64), :], on[:, 256:512])

            def do_a2a(b):
                nc.gpsimd.collective_compute(
                    "AllToAll", mybir.AluOpType.bypass,
                    replica_groups=[list(range(N_CORES))],
                    ins=[a2a_in[b].opt()], outs=[a2a_out[b].opt()])

            def oproj_load(b):
                attf = sp.tile([128, 16, 256], f32, tag="attf", bufs=1)
                nc.sync.dma_start(
                    attf[:], a2a_out[b][:].rearrange("(k p) n -> p k n", p=128))
                nc.vector.tensor_copy(attb[:], attf[:].rearrange("p k n -> p (k n)"))

            def oproj_ms(b, ms):
                for m in ms:
                    wostrip = wp.tile([128, 2048], bf, tag="wo")
                    nc.sync.dma_start(wostrip[:], wo_sb[:, ds(m * 2048, 2048)])
                    psf = pproj.tile([128, 256], f32, tag="proj", name="psf")
                    for k in range(16):
                        nc.tensor.matmul(psf[:], wostrip[:, ts(k, 128)], attb[:, ts(k, 256)],
                                         start=(k == 0), stop=(k == 15))
                    ofin = sp.tile([128, 256], f32, tag="ofin")
                    nc.vector.tensor_copy(ofin[:], psf[:])
                    nc.sync.dma_start(out_ext[ts(m, 128), ds(b * 256, 256)], ofin[:])

            # ---- emission schedule ----
            for r in range(4):
                proj_chunk(r)
            blocks = [(hp, qs) for qs in range(4) for hp in range(2)]
            for i, (hp, qs) in enumerate(blocks):
                attn_block(0, hp, qs)
                if i in (0, 2, 4, 6):
                    proj_chunk(4 + i // 2)
            do_a2a(0)
            for i, (hp, qs) in enumerate(blocks):
                attn_block(1, hp, qs)
                if i == 1:
                    oproj_load(0)
                elif i >= 2:
                    oproj_ms(0, range(2 * (i - 2), 2 * (i - 1)))
            do_a2a(1)
            oproj_ms(0, range(12, 16))
            oproj_load(1)
            oproj_ms(1, range(16))

    nc.compile()
    return nc


def _host_prep(x, freqs_cos, freqs_sin, wq, wk, wv, wo, q_scale, k_scale):
    bfd = ml_dtypes.bfloat16
    perm = np.concatenate([np.arange(0, 64, 2), np.arange(1, 64, 2)])

    xT = np.ascontiguousarray(x.reshape(ROWS, D).T)
    x_sb = np.ascontiguousarray(
        xT.reshape(NKT, 128, ROWS).transpose(1, 0, 2).reshape(128, NKT * ROWS)
    ).astype(bfd)

    ct = np.concatenate([freqs_cos.T, freqs_cos.T], axis=1)   # [32, 4096]
    st = np.concatenate([freqs_sin.T, freqs_sin.T], axis=1)
    cos_sb = np.ascontiguousarray(np.tile(ct, (4, 1))).astype(np.float32)
    sin_sb = np.ascontiguousarray(np.concatenate([-st, st, -st, st], 0)).astype(np.float32)

    r = np.arange(128)[:, None]
    c = np.arange(896)[None, :]
    msk_sb = (c >= r + 384).astype(bfd)
    ind_sb = np.zeros((128, 2), bfd)
    ind_sb[0:64, 0] = 1
    ind_sb[64:128, 1] = 1
    idn_sb = np.eye(64, dtype=bfd)
    sc_sb = np.stack([q_scale[perm], k_scale[perm]], axis=1).astype(np.float32)

    woT = wo.T.astype(np.float32)  # [hdim, dout]
    wo_sb = np.ascontiguousarray(
        woT.reshape(16, 128, 16, 128).transpose(1, 2, 0, 3).reshape(128, 16 * 16 * 128)
    ).astype(bfd)

    shared = dict(x_sb=x_sb, cos_sb=cos_sb, sin_sb=sin_sb, msk_sb=msk_sb,
                  ind_sb=ind_sb, idn_sb=idn_sb, sc_sb=sc_sb, wo_sb=wo_sb)

    in_maps = []
    for cc in range(N_CORES):
        wq_c = wq[cc * 256:(cc + 1) * 256].reshape(4, 64, D)[:, perm].reshape(256, D)
        wqT = wq_c.T  # [D, 256]
        wq_core = np.ascontiguousarray(
            wqT.reshape(NKT, 128, 2, 128).transpose(1, 0, 2, 3).reshape(128, NKT * 256)
        ).astype(bfd)
        wk_c = wk[cc * 64:(cc + 1) * 64][perm]
        wkTd = np.concatenate([wk_c, wk_c], 0).T  # [D, 128]
        wk_core = np.ascontiguousarray(
            wkTd.reshape(NKT, 128, 128).transpose(1, 0, 2).reshape(128, NKT * 128)
        ).astype(bfd)
        wvT = wv[cc * 64:(cc + 1) * 64].T  # [D, 64]
        wv_core = np.ascontiguousarray(
            wvT.reshape(NKT, 128, 64).transpose(1, 0, 2).reshape(128, NKT * 64)
        ).astype(bfd)
        in_maps.append(dict(shared, wq_sb=wq_core, wk_sb=wk_core, wv_sb=wv_core))
    return in_maps


def kernel(x, freqs_cos, freqs_sin, wq, wk, wv, wo, q_scale, k_scale, _trace=False):
    from concourse.bass_utils import run_bass_kernel_spmd

    if "nc" not in _cache:
        _cache["nc"] = _build()
    nc = _cache["nc"]

    args = [np.asarray(a, dtype=np.float32) for a in
            (x, freqs_cos, freqs_sin, wq, wk, wv, wo, q_scale, k_scale)]
    in_maps = _host_prep(*args)
    res = run_bass_kernel_spmd(nc, in_maps, list(range(N_CORES)), trace=_trace)
    out = np.zeros((B, S, D), np.float32)
    for cc in range(N_CORES):
        oc = res.results[cc]["out"]  # [2048, 512]
        for b in range(B):
            out[b, 256 * cc:256 * (cc + 1), :] = oc[:, 256 * b:256 * (b + 1)].T
    if _trace:
        return out, res
    return out


# revision 9
# speedup vs baseline: 1.0608x; 1.0608x over previous
"""Distributed GQA attention block (dense_transformer) for 8 TRN2 NeuronCores.

Sharding: Megatron-style head sharding for QKV+attention (each core owns 4 Q
heads / 1 KV head), Ulysses-style AllToAll to switch to sequence sharding for
the output projection (each core owns 256 rows per batch).

Layouts (per core, SPMD identical graph):
  - All activations kept transposed: QT/KT [head_dim, rows] so scores come out
    as S^T [k, q] and softmax reduces over the partition axis via matmul with a
    fused ones-column in V (denominator for free).
  - RoPE handled by permuting head dims (evens|odds) in the weights on the
    host, so rotation pairs are partition halves: out = qn*cos + swap(qn)*sin±.
  - RMSNorm partition-reduction via indicator matmul; rsqrt via ACT ln/exp
    (same ACT table set as softmax exp -> zero table switches).
Compute in bf16 on the TensorEngine (f32 accumulation), f32 softmax.
"""
import sys

if '/opt/trn_rl_repo' not in sys.path:
    sys.path.insert(0, '/opt/trn_rl_repo')

import numpy as np
import ml_dtypes

N_CORES = 8
B, S, D = 2, 2048, 2048
DH = 64
HLOC = 4            # Q heads per core
ROWS = B * S        # 4096
NKT = D // 128      # 16 contraction tiles
RC = 512            # row chunk
NCHUNK = ROWS // RC # 8
EPS = 1e-6

_cache = {}


def _build():
    import concourse.mybir as mybir
    import concourse.tile as tile
    from concourse import bacc
    from concourse.bass import ts, ds

    f32 = mybir.dt.float32
    bf = mybir.dt.bfloat16
    AF = mybir.ActivationFunctionType
    MUL = mybir.AluOpType.mult

    nc = bacc.Bacc()
    x_sb = nc.declare_dram_parameter("x_sb", [128, NKT * ROWS], bf, isOutput=False)
    wq_sb = nc.declare_dram_parameter("wq_sb", [128, NKT * 2 * 128], bf, isOutput=False)
    wk_sb = nc.declare_dram_parameter("wk_sb", [128, NKT * 128], bf, isOutput=False)
    wv_sb = nc.declare_dram_parameter("wv_sb", [128, NKT * 64], bf, isOutput=False)
    wo_sb = nc.declare_dram_parameter("wo_sb", [128, 16 * 16 * 128], bf, isOutput=False)
    cos_sb = nc.declare_dram_parameter("cos_sb", [128, ROWS], f32, isOutput=False)
    sin_sb = nc.declare_dram_parameter("sin_sb", [128, ROWS], f32, isOutput=False)
    msk_sb = nc.declare_dram_parameter("msk_sb", [128, 896], bf, isOutput=False)
    ind_sb = nc.declare_dram_parameter("ind_sb", [128, 2], bf, isOutput=False)
    indt_sb = nc.declare_dram_parameter("indt_sb", [2, 128], bf, isOutput=False)
    scp_sb = nc.declare_dram_parameter("scp_sb", [128, 2], f32, isOutput=False)
    idn_sb = nc.declare_dram_parameter("idn_sb", [64, 64], bf, isOutput=False)
    sc_sb = nc.declare_dram_parameter("sc_sb", [64, 2], f32, isOutput=False)
    out_ext = nc.declare_dram_parameter("out", [D, 512], f32, isOutput=True)

    with tile.TileContext(nc) as tc:
        with (
            tc.tile_pool(name="cp", bufs=1) as cp,
            tc.tile_pool(name="xp", bufs=2) as xp,
            tc.tile_pool(name="wp", bufs=3) as wp,
            tc.tile_pool(name="sp", bufs=2) as sp,
            tc.tile_pool(name="ep", bufs=4) as ep,
            tc.tile_pool(name="dram", bufs=1, space="DRAM") as dram,
            tc.tile_pool(name="pmm", bufs=4, space="PSUM") as pmm,
            tc.tile_pool(name="po", bufs=2, space="PSUM") as po,
            tc.tile_pool(name="psm", bufs=2, space="PSUM") as psm,
        ):
            # ---- persistent constants ----
            wq = cp.tile([128, NKT * 2 * 128], bf)
            for i in range(4):
                nc.sync.dma_start(wq[:, ds(i * 1024, 1024)], wq_sb[:, ds(i * 1024, 1024)])
            wkt = cp.tile([128, NKT * 128], bf)
            nc.sync.dma_start(wkt[:], wk_sb[:])
            wvt = cp.tile([128, NKT * 64], bf)
            nc.sync.dma_start(wvt[:], wv_sb[:])
            msk = cp.tile([128, 896], bf)
            nc.sync.dma_start(msk[:], msk_sb[:])
            ind = cp.tile([128, 2], bf)
            nc.sync.dma_start(ind[:], ind_sb[:])
            indt = cp.tile([2, 128], bf)
            nc.sync.dma_start(indt[:], indt_sb[:])
            scp = cp.tile([128, 2], f32)
            nc.sync.dma_start(scp[:], scp_sb[:])
            idn = cp.tile([64, 64], bf)
            nc.sync.dma_start(idn[:], idn_sb[:])
            sc2 = cp.tile([64, 2], f32)
            nc.sync.dma_start(sc2[:], sc_sb[:])
            epsc = cp.tile([128, 1], f32)
            nc.gpsimd.memset(epsc[:], EPS)
            ones64 = cp.tile([1, 64], bf)
            nc.gpsimd.memset(ones64[:], 1.0)

            QTn = [cp.tile([128, ROWS], bf, name=f"qtn{i}") for i in range(2)]
            KTd = [cp.tile([128, S], bf, name=f"ktd{b}") for b in range(B)]
            Vb1 = [cp.tile([128, 16 * 65], bf, name=f"vb{b}") for b in range(B)]
            attb = cp.tile([128, 16 * 256], bf)

            a2a_in = [dram.tile([2048, 256], bf, name=f"a2ain{b}") for b in range(B)]
            a2a_out = [dram.tile([2048, 256], bf, name=f"a2aout{b}") for b in range(B)]

            # ---- norm + rope on a projection psum tile ----
            def norm_rope(ps, dst_ap, cosc, sinc, sc_col, dup):
                sq = sp.tile([128, RC], bf, tag="sq")
                nc.scalar.activation(sq[:], ps[:], AF.Square)
                ssp = psm.tile([2, RC], f32, tag="small", name="ssp")
                nc.tensor.matmul(ssp[:], ind[:], sq[:], start=True, stop=True)
                lg = sp.tile([2, RC], f32, tag="lg", bufs=4)
                nc.scalar.activation(lg[:], ssp[:], AF.Ln, scale=1.0 / 64, bias=epsc[0:2, :])
                rstd = sp.tile([2, RC], bf, tag="rstd", bufs=4)
                nc.scalar.activation(rstd[:], lg[:], AF.Exp, scale=-0.5)
                bcp = psm.tile([128, RC], f32, tag="small", name="bcpn")
                nc.tensor.matmul(bcp[:], indt[:], rstd[:], start=True, stop=True)
                bcs = sp.tile([128, RC], f32, tag="bcs")
                nc.vector.tensor_scalar(
                    out=bcs[:], in0=bcp[:], scalar1=scp[:, sc_col:sc_col + 1],
                    scalar2=None, op0=MUL)
                qn = sp.tile([128, RC], f32, tag="qn")
                nc.vector.tensor_mul(qn[0:64, :], ps[0:64, :], bcs[0:64, :])
                nc.vector.tensor_mul(qn[64:128, :], ps[64:128, :], bcs[64:128, :])
                swp = sp.tile([128, RC], f32, tag="swp")
                for g in range(4):
                    nc.vector.tensor_mul(swp[ts(g, 32), :], ps[ts(g ^ 1, 32), :],
                                         bcs[ts(g ^ 1, 32), :])
                nc.vector.tensor_mul(qn[:], qn[:], cosc[:])
                nc.vector.tensor_mul(swp[:], swp[:], sinc[:])
                nc.vector.tensor_add(dst_ap, qn[:], swp[:])

            # ---- one row-chunk of projections ----
            def proj_chunk(r):
                b, sl = r // 4, r % 4
                xt = xp.tile([128, NKT, RC], bf, tag="xt")
                nc.sync.dma_start(
                    xt[:], x_sb[:].rearrange("p (k q) -> p k q", k=NKT)[:, :, ds(r * RC, RC)])
                cosc = sp.tile([128, RC], f32, tag="cos")
                nc.sync.dma_start(cosc[:], cos_sb[:, ds(r * RC, RC)])
                sinc = sp.tile([128, RC], f32, tag="sin")
                nc.sync.dma_start(sinc[:], sin_sb[:, ds(r * RC, RC)])
                for hp in range(2):
                    psq = pmm.tile([128, RC], f32, tag="mm", name=f"psq{hp}")
                    for k in range(NKT):
                        nc.tensor.matmul(psq[:], wq[:, ds((k * 2 + hp) * 128, 128)],
                                         xt[:, k, :], start=(k == 0), stop=(k == NKT - 1))
                    norm_rope(psq, QTn[hp][:, ds(r * RC, RC)], cosc, sinc, 0, False)
                psk = pmm.tile([128, RC], f32, tag="mm")
                for k in range(NKT):
                    nc.tensor.matmul(psk[:], wkt[:, ts(k, 128)], xt[:, k, :],
                                     start=(k == 0), stop=(k == NKT - 1))
                norm_rope(psk, KTd[b][:, ds(sl * RC, RC)], cosc, sinc, 1, True)
                psv = pmm.tile([64, RC], f32, tag="mm")
                for k in range(NKT):
                    nc.tensor.matmul(psv[:], wvt[:, ts(k, 64)], xt[:, k, :],
                                     start=(k == 0), stop=(k == NKT - 1))
                vtmp = sp.tile([64, RC], bf, tag="vtmp")
                nc.vector.tensor_copy(vtmp[:], psv[:])
                for t4 in range(4):
                    tp = psm.tile([128, 64], bf, tag="small", name="tp")
                    nc.tensor.transpose(tp[:], vtmp[:, ts(t4, 128)], idn[:])
                    gt = sl * 4 + t4
                    nc.vector.tensor_copy(Vb1[b][:, ds(gt * 65, 64)], tp[:])
                    nc.vector.memset(Vb1[b][:, ds(gt * 65 + 64, 1)], 1.0)

            # ---- one attention block: batch b, head-pair hp, q-slice qs ----
            def attn_block(b, hp, qs):
                psO = [po.tile([65, RC], f32, tag="o", name=f"psO{t}") for t in range(2)]
                jmax = qs * 4 + 3
                for j in range(jmax + 1):
                    dj = j - qs * 4
                    p = dj * 128 if dj >= 0 else 0
                    N = RC - p
                    qb = b * S + qs * RC + p
                    psS = [pmm.tile([128, RC], f32, tag="mm", name=f"psS{t}")
                           for t in range(2)]
                    nc.tensor.matmul(psS[0][:, 0:N], KTd[b][0:64, ts(j, 128)],
                                     QTn[hp][0:64, ds(qb, N)], start=True, stop=True,
                                     tile_position=(0, 0))
                    nc.tensor.matmul(psS[1][:, 0:N], KTd[b][64:128, ts(j, 128)],
                                     QTn[hp][64:128, ds(qb, N)], start=True, stop=True,
                                     tile_position=(64, 0))
                    for t in range(2):
                        E = ep.tile([128, RC], bf, tag="E")
                        nc.scalar.activation(E[:, p:RC], psS[t][:, 0:N], AF.Exp, scale=0.125)
                        if dj >= 0:
                            nc.vector.tensor_mul(E[:, ds(p, 128)], E[:, ds(p, 128)],
                                                 msk[:, ds(384, 128)])
                        nc.tensor.matmul(psO[t][:, p:RC], Vb1[b][:, ds(j * 65, 65)],
                                         E[:, p:RC], start=(j == 0), stop=(j == jmax))
                for t in range(2):
                    hl = 2 * hp + t
                    recip = sp.tile([1, RC], bf, tag="recip")
                    with nc.allow_low_precision(reason="softmax denom recip feeds bf16 bcast matmul"):
                        nc.vector.reciprocal(recip[:], psO[t][64:65, :])
                    bcp = psm.tile([64, RC], f32, tag="small", name="bcp")
                    nc.tensor.matmul(bcp[:], ones64[:], recip[:], start=True, stop=True)
                    rbco = sp.tile([64, RC], f32, tag="rbco")
                    nc.vector.tensor_copy(rbco[:], bcp[:])
                    on = sp.tile([64, RC], bf, tag="on")
                    nc.vector.tensor_mul(on[:], psO[t][0:64, :], rbco[:])
                    nc.sync.dma_start(
                        a2a_in[b][ds(256 * (2 * qs) + hl * 64, 64), :], on[:, 0:256])
                    nc.sync.dma_start(
                        a2a_in[b][ds(256 * (2 * qs + 1) + hl * 64, 64), :], on[:, 256:512])

            def do_a2a(b):
                nc.gpsimd.collective_compute(
                    "AllToAll", mybir.AluOpType.bypass,
                    replica_groups=[list(range(N_CORES))],
                    ins=[a2a_in[b].opt()], outs=[a2a_out[b].opt()])

            def oproj_load(b):
                nc.sync.dma_start(
                    attb[:].rearrange("p (k n) -> p k n", k=16),
                    a2a_out[b][:].rearrange("(k p) n -> p k n", p=128))

            def oproj_ms(b, ms):
                for m in ms:
                    wostrip = wp.tile([128, 2048], bf, tag="wo")
                    nc.sync.dma_start(wostrip[:], wo_sb[:, ds(m * 2048, 2048)])
                    psf = pmm.tile([128, 256], f32, tag="mm", name="psf")
                    for k in range(16):
                        nc.tensor.matmul(psf[:], wostrip[:, ts(k, 128)], attb[:, ts(k, 256)],
                                         start=(k == 0), stop=(k == 15))
                    ofin = sp.tile([128, 256], f32, tag="ofin")
                    nc.vector.tensor_copy(ofin[:], psf[:])
                    nc.sync.dma_start(out_ext[ts(m, 128), ds(b * 256, 256)], ofin[:])

            # ---- emission schedule ----
            for r in range(4):
                proj_chunk(r)
            blocks = [(hp, qs) for qs in range(4) for hp in range(2)]
            for i, (hp, qs) in enumerate(blocks):
                attn_block(0, hp, qs)
                if i in (0, 2, 4, 6):
                    proj_chunk(4 + i // 2)
            do_a2a(0)
            for i, (hp, qs) in enumerate(blocks):
                attn_block(1, hp, qs)
                if i == 1:
                    oproj_load(0)
                elif 2 <= i <= 6:
                    oproj_ms(0, [i - 2])
            do_a2a(1)
            oproj_ms(0, range(5, 16))
            oproj_load(1)
            oproj_ms(1, range(16))

    nc.compile()
    return nc


def _host_prep(x, freqs_cos, freqs_sin, wq, wk, wv, wo, q_scale, k_scale):
    bfd = ml_dtypes.bfloat16
    perm = np.concatenate([np.arange(0, 64, 2), np.arange(1, 64, 2)])

    xT = np.ascontiguousarray(x.reshape(ROWS, D).T)
    x_sb = np.ascontiguousarray(
        xT.reshape(NKT, 128, ROWS).transpose(1, 0, 2).reshape(128, NKT * ROWS)
    ).astype(bfd)

    ct = np.concatenate([freqs_cos.T, freqs_cos.T], axis=1)   # [32, 4096]
    st = np.concatenate([freqs_sin.T, freqs_sin.T], axis=1)
    cos_sb = np.ascontiguousarray(np.tile(ct, (4, 1))).astype(np.float32)
    sin_sb = np.ascontiguousarray(np.concatenate([-st, st, -st, st], 0)).astype(np.float32)

    r = np.arange(128)[:, None]
    c = np.arange(896)[None, :]
    msk_sb = (c >= r + 384).astype(bfd)
    ind_sb = np.zeros((128, 2), bfd)
    ind_sb[0:64, 0] = 1
    ind_sb[64:128, 1] = 1
    indt_sb = np.ascontiguousarray(ind_sb.T)
    scp_sb = np.stack([np.tile(q_scale[perm], 2), np.tile(k_scale[perm], 2)],
                      axis=1).astype(np.float32)
    idn_sb = np.eye(64, dtype=bfd)
    sc_sb = np.stack([q_scale[perm], k_scale[perm]], axis=1).astype(np.float32)

    woT = wo.T.astype(np.float32)  # [hdim, dout]
    wo_sb = np.ascontiguousarray(
        woT.reshape(16, 128, 16, 128).transpose(1, 2, 0, 3).reshape(128, 16 * 16 * 128)
    ).astype(bfd)

    shared = dict(x_sb=x_sb, cos_sb=cos_sb, sin_sb=sin_sb, msk_sb=msk_sb,
                  ind_sb=ind_sb, indt_sb=indt_sb, scp_sb=scp_sb,
                  idn_sb=idn_sb, sc_sb=sc_sb, wo_sb=wo_sb)

    in_maps = []
    for cc in range(N_CORES):
        wq_c = wq[cc * 256:(cc + 1) * 256].reshape(4, 64, D)[:, perm].reshape(256, D)
        wqT = wq_c.T  # [D, 256]
        wq_core = np.ascontiguousarray(
            wqT.reshape(NKT, 128, 2, 128).transpose(1, 0, 2, 3).reshape(128, NKT * 256)
        ).astype(bfd)
        wk_c = wk[cc * 64:(cc + 1) * 64][perm]
        wkTd = np.concatenate([wk_c, wk_c], 0).T  # [D, 128]
        wk_core = np.ascontiguousarray(
            wkTd.reshape(NKT, 128, 128).transpose(1, 0, 2).reshape(128, NKT * 128)
        ).astype(bfd)
        wvT = wv[cc * 64:(cc + 1) * 64].T  # [D, 64]
        wv_core = np.ascontiguousarray(
            wvT.reshape(NKT, 128, 64).transpose(1, 0, 2).reshape(128, NKT * 64)
        ).astype(bfd)
        in_maps.append(dict(shared, wq_sb=wq_core, wk_sb=wk_core, wv_sb=wv_core))
    return in_maps


def kernel(x, freqs_cos, freqs_sin, wq, wk, wv, wo, q_scale, k_scale, _trace=False):
    from concourse.bass_utils import run_bass_kernel_spmd

    if "nc" not in _cache:
        _cache["nc"] = _build()
    nc = _cache["nc"]

    args = [np.asarray(a, dtype=np.float32) for a in
            (x, freqs_cos, freqs_sin, wq, wk, wv, wo, q_scale, k_scale)]
    in_maps = _host_prep(*args)
    res = run_bass_kernel_spmd(nc, in_maps, list(range(N_CORES)), trace=_trace)
    out = np.zeros((B, S, D), np.float32)
    for cc in range(N_CORES):
        oc = res.results[cc]["out"]  # [2048, 512]
        for b in range(B):
            out[b, 256 * cc:256 * (cc + 1), :] = oc[:, 256 * b:256 * (b + 1)].T
    if _trace:
        return out, res
    return out


# revision 12
# speedup vs baseline: 1.2584x; 1.1863x over previous
"""Distributed GQA attention block (dense_transformer) for 8 TRN2 NeuronCores.

Sharding: Megatron-style head sharding for QKV+attention (each core owns 4 Q
heads / 1 KV head), Ulysses-style AllToAll to switch to sequence sharding for
the output projection (each core owns 256 rows per batch).

Layouts (per core, SPMD identical graph):
  - All activations kept transposed: QT/KT [head_dim, rows] so scores come out
    as S^T [k, q] and softmax reduces over the partition axis via matmul with a
    fused ones-column in V (denominator for free).
  - RoPE handled by permuting head dims (evens|odds) in the weights on the
    host, so rotation pairs are partition halves: out = qn*cos + swap(qn)*sin±.
  - RMSNorm partition-reduction via indicator matmul; rsqrt via ACT ln/exp
    (same ACT table set as softmax exp -> zero table switches).
Compute in bf16 on the TensorEngine (f32 accumulation), f32 softmax.
"""
import sys

if '/opt/trn_rl_repo' not in sys.path:
    sys.path.insert(0, '/opt/trn_rl_repo')

import numpy as np
import ml_dtypes

N_CORES = 8
B, S, D = 2, 2048, 2048
DH = 64
HLOC = 4            # Q heads per core
ROWS = B * S        # 4096
NKT = D // 128      # 16 contraction tiles
RC = 512            # row chunk
NCHUNK = ROWS // RC # 8
EPS = 1e-6

_cache = {}


def _build():
    import concourse.mybir as mybir
    import concourse.tile as tile
    from concourse import bacc
    from concourse.bass import ts, ds

    f32 = mybir.dt.float32
    bf = mybir.dt.bfloat16
    AF = mybir.ActivationFunctionType
    MUL = mybir.AluOpType.mult

    nc = bacc.Bacc()
    x_sb = nc.declare_dram_parameter("x_sb", [128, NKT * ROWS], bf, isOutput=False)
    wq_sb = nc.declare_dram_parameter("wq_sb", [128, NKT * 2 * 128], bf, isOutput=False)
    wk_sb = nc.declare_dram_parameter("wk_sb", [128, NKT * 128], bf, isOutput=False)
    wv_sb = nc.declare_dram_parameter("wv_sb", [128, NKT * 64], bf, isOutput=False)
    wo_sb = nc.declare_dram_parameter("wo_sb", [128, 16 * 16 * 128], bf, isOutput=False)
    cos_sb = nc.declare_dram_parameter("cos_sb", [128, ROWS], bf, isOutput=False)
    sin_sb = nc.declare_dram_parameter("sin_sb", [128, ROWS], bf, isOutput=False)
    msk_sb = nc.declare_dram_parameter("msk_sb", [128, 896], bf, isOutput=False)
    ind_sb = nc.declare_dram_parameter("ind_sb", [128, 2], bf, isOutput=False)
    indt_sb = nc.declare_dram_parameter("indt_sb", [2, 128], bf, isOutput=False)
    scp_sb = nc.declare_dram_parameter("scp_sb", [128, 2], f32, isOutput=False)
    idn_sb = nc.declare_dram_parameter("idn_sb", [64, 64], bf, isOutput=False)
    psw_sb = nc.declare_dram_parameter("psw_sb", [128, 128], bf, isOutput=False)
    sc_sb = nc.declare_dram_parameter("sc_sb", [64, 2], f32, isOutput=False)
    out_ext = nc.declare_dram_parameter("out", [D, 512], f32, isOutput=True)

    with tile.TileContext(nc) as tc:
        with (
            tc.tile_pool(name="cp", bufs=1) as cp,
            tc.tile_pool(name="xp", bufs=2) as xp,
            tc.tile_pool(name="wp", bufs=4) as wp,
            tc.tile_pool(name="sp", bufs=2) as sp,
            tc.tile_pool(name="ep", bufs=4) as ep,
            tc.tile_pool(name="dram", bufs=1, space="DRAM") as dram,
            tc.tile_pool(name="pmm", bufs=2, space="PSUM") as pmm,
            tc.tile_pool(name="po", bufs=2, space="PSUM") as po,
            tc.tile_pool(name="psm", bufs=2, space="PSUM") as psm,
        ):
            # ---- persistent constants ----
            wq = cp.tile([128, NKT * 2 * 128], bf)
            for i in range(4):
                nc.sync.dma_start(wq[:, ds(i * 1024, 1024)], wq_sb[:, ds(i * 1024, 1024)])
            wkt = cp.tile([128, NKT * 128], bf)
            nc.sync.dma_start(wkt[:], wk_sb[:])
            wvt = cp.tile([128, NKT * 64], bf)
            nc.sync.dma_start(wvt[:], wv_sb[:])
            msk = cp.tile([128, 896], bf)
            nc.sync.dma_start(msk[:], msk_sb[:])
            ind = cp.tile([128, 2], bf)
            nc.sync.dma_start(ind[:], ind_sb[:])
            indt = cp.tile([2, 128], bf)
            nc.sync.dma_start(indt[:], indt_sb[:])
            scp = cp.tile([128, 2], f32)
            nc.sync.dma_start(scp[:], scp_sb[:])
            idn = cp.tile([64, 64], bf)
            nc.sync.dma_start(idn[:], idn_sb[:])
            sc2 = cp.tile([64, 2], f32)
            nc.sync.dma_start(sc2[:], sc_sb[:])
            epsc = cp.tile([128, 1], f32)
            nc.gpsimd.memset(epsc[:], EPS)
            ones64 = cp.tile([1, 64], bf)
            nc.gpsimd.memset(ones64[:], 1.0)
            pswap = cp.tile([128, 128], bf)
            nc.sync.dma_start(pswap[:], psw_sb[:])

            QTn = [cp.tile([128, ROWS], bf, name=f"qtn{i}") for i in range(2)]
            KTd = [cp.tile([128, S], bf, name=f"ktd{b}") for b in range(B)]
            Vb1 = [cp.tile([128, 16 * 65], bf, name=f"vb{b}") for b in range(B)]
            attb = cp.tile([128, 16 * 256], bf)

            a2a_in = [dram.tile([2048, 256], bf, name=f"a2ain{b}") for b in range(B)]
            a2a_out = [dram.tile([2048, 256], bf, name=f"a2aout{b}") for b in range(B)]

            # ---- norm + rope on a projection psum tile ----
            def norm_rope(ps, dst_ap, cosc, sinc, sc_col, dup):
                sq = sp.tile([128, RC], bf, tag="sq")
                nc.scalar.activation(sq[:], ps[:], AF.Square)
                ssp = psm.tile([2, RC], f32, tag="small", name="ssp")
                nc.tensor.matmul(ssp[:], ind[:], sq[:], start=True, stop=True)
                lg = sp.tile([2, RC], f32, tag="lg", bufs=4)
                nc.scalar.activation(lg[:], ssp[:], AF.Ln, scale=1.0 / 64, bias=epsc[0:2, :])
                rstd = sp.tile([2, RC], bf, tag="rstd", bufs=4)
                nc.scalar.activation(rstd[:], lg[:], AF.Exp, scale=-0.5)
                bcp = psm.tile([128, RC], f32, tag="small", name="bcpn")
                nc.tensor.matmul(bcp[:], indt[:], rstd[:], start=True, stop=True)
                bcs = sp.tile([128, RC], f32, tag="bcs")
                nc.vector.tensor_scalar(
                    out=bcs[:], in0=bcp[:], scalar1=scp[:, sc_col:sc_col + 1],
                    scalar2=None, op0=MUL)
                qn = sp.tile([128, RC], bf, tag="qn")
                nc.vector.tensor_mul(qn[:], ps[:], bcs[:])
                swps = psm.tile([128, RC], f32, tag="small", name="swps")
                nc.tensor.matmul(swps[:], pswap[:], qn[:], start=True, stop=True)
                swp = sp.tile([128, RC], bf, tag="swp")
                nc.vector.tensor_mul(swp[:], swps[:], sinc[:])
                nc.vector.tensor_mul(qn[:], qn[:], cosc[:])
                nc.vector.tensor_add(dst_ap, qn[:], swp[:])

            # ---- one row-chunk of projections ----
            def proj_chunk(r):
                b, sl = r // 4, r % 4
                xt = xp.tile([128, NKT, RC], bf, tag="xt")
                nc.sync.dma_start(
                    xt[:], x_sb[:].rearrange("p (k q) -> p k q", k=NKT)[:, :, ds(r * RC, RC)])
                cosc = sp.tile([128, RC], bf, tag="cos")
                nc.sync.dma_start(cosc[:], cos_sb[:, ds(r * RC, RC)])
                sinc = sp.tile([128, RC], bf, tag="sin")
                nc.sync.dma_start(sinc[:], sin_sb[:, ds(r * RC, RC)])
                for hp in range(2):
                    psq = pmm.tile([128, RC], f32, tag="mm", padded_shape=[128, 1024], name=f"psq{hp}")
                    for k in range(NKT):
                        nc.tensor.matmul(psq[:], wq[:, ds((k * 2 + hp) * 128, 128)],
                                         xt[:, k, :], start=(k == 0), stop=(k == NKT - 1))
                    norm_rope(psq, QTn[hp][:, ds(r * RC, RC)], cosc, sinc, 0, False)
                psk = pmm.tile([128, RC], f32, tag="mm", padded_shape=[128, 1024])
                for k in range(NKT):
                    nc.tensor.matmul(psk[:], wkt[:, ts(k, 128)], xt[:, k, :],
                                     start=(k == 0), stop=(k == NKT - 1))
                norm_rope(psk, KTd[b][:, ds(sl * RC, RC)], cosc, sinc, 1, True)
                psv = pmm.tile([64, RC], f32, tag="mm", padded_shape=[128, 1024])
                for k in range(NKT):
                    nc.tensor.matmul(psv[:], wvt[:, ts(k, 64)], xt[:, k, :],
                                     start=(k == 0), stop=(k == NKT - 1))
                vtmp = sp.tile([64, RC], bf, tag="vtmp")
                nc.vector.tensor_copy(vtmp[:], psv[:])
                for t4 in range(4):
                    tp = psm.tile([128, 64], bf, tag="small", name="tp")
                    nc.tensor.transpose(tp[:], vtmp[:, ts(t4, 128)], idn[:])
                    gt = sl * 4 + t4
                    nc.vector.tensor_copy(Vb1[b][:, ds(gt * 65, 64)], tp[:])
                    nc.vector.memset(Vb1[b][:, ds(gt * 65 + 64, 1)], 1.0)

            # ---- one attention block: batch b, head-pair hp, q-slice qs ----
            def attn_block(b, hp, qs):
                psO = [po.tile([65, RC], f32, tag="o", name=f"psO{t}") for t in range(2)]
                jmax = qs * 4 + 3

                def scores(j):
                    dj = j - qs * 4
                    p = dj * 128 if dj >= 0 else 0
                    N = RC - p
                    qb = b * S + qs * RC + p
                    psS = pmm.tile([128, 2 * RC], f32, tag="mm", name="psS")
                    nc.tensor.matmul(psS[:, 0:N], KTd[b][0:64, ts(j, 128)],
                                     QTn[hp][0:64, ds(qb, N)], start=True, stop=True,
                                     tile_position=(0, 0))
                    nc.tensor.matmul(psS[:, RC:RC + N], KTd[b][64:128, ts(j, 128)],
                                     QTn[hp][64:128, ds(qb, N)], start=True, stop=True,
                                     tile_position=(64, 0))
                    E = ep.tile([128, 2 * RC], bf, tag="E")
                    e3 = E[:].rearrange("p (t q) -> p t q", t=2)
                    s3 = psS[:].rearrange("p (t q) -> p t q", t=2)
                    nc.scalar.activation(e3[:, :, p:RC], s3[:, :, 0:N], AF.Exp, scale=0.125)
                    if dj >= 0:
                        nc.gpsimd.affine_select(
                            out=e3[:, :, ds(p, 128)], in_=e3[:, :, ds(p, 128)],
                            pattern=[[0, 2], [1, 128]], compare_op=mybir.AluOpType.is_ge,
                            fill=0.0, base=0, channel_multiplier=-1)
                    return p, E

                def av(j, pE):
                    p, E = pE
                    for t in range(2):
                        nc.tensor.matmul(psO[t][:, p:RC], Vb1[b][:, ds(j * 65, 65)],
                                         E[:, ds(t * RC + p, RC - p)],
                                         start=(j == 0), stop=(j == jmax))

                pend = {0: scores(0)}
                for j in range(1, jmax + 1):
                    pend[j] = scores(j)
                    av(j - 1, pend.pop(j - 1))
                av(jmax, pend.pop(jmax))
                for t in range(2):
                    hl = 2 * hp + t
                    recip = sp.tile([1, RC], bf, tag="recip")
                    with nc.allow_low_precision(reason="softmax denom recip feeds bf16 bcast matmul"):
                        nc.vector.reciprocal(recip[:], psO[t][64:65, :])
                    bcp = psm.tile([64, RC], f32, tag="small", name="bcp")
                    nc.tensor.matmul(bcp[:], ones64[:], recip[:], start=True, stop=True)
                    rbco = sp.tile([64, RC], f32, tag="rbco")
                    nc.vector.tensor_copy(rbco[:], bcp[:])
                    on = sp.tile([64, RC], bf, tag="on")
                    nc.vector.tensor_mul(on[:], psO[t][0:64, :], rbco[:])
                    nc.sync.dma_start(
                        a2a_in[b][ds(256 * (2 * qs) + hl * 64, 64), :], on[:, 0:256])
                    nc.sync.dma_start(
                        a2a_in[b][ds(256 * (2 * qs + 1) + hl * 64, 64), :], on[:, 256:512])

            def do_a2a(b):
                nc.gpsimd.collective_compute(
                    "AllToAll", mybir.AluOpType.bypass,
                    replica_groups=[list(range(N_CORES))],
                    ins=[a2a_in[b].opt()], outs=[a2a_out[b].opt()])

            def oproj_load(b):
                nc.sync.dma_start(
                    attb[:].rearrange("p (k n) -> p k n", k=16),
                    a2a_out[b][:].rearrange("(k p) n -> p k n", p=128))

            def oproj_ms(b, mpairs):
                for mp in mpairs:
                    psf = pmm.tile([128, RC], f32, tag="mm", padded_shape=[128, 1024], name="psf")
                    for mi, m in enumerate((2 * mp, 2 * mp + 1)):
                        wostrip = wp.tile([128, 2048], bf, tag="wo")
                        nc.sync.dma_start(wostrip[:], wo_sb[:, ds(m * 2048, 2048)])
                        for k in range(16):
                            nc.tensor.matmul(psf[:, ds(mi * 256, 256)],
                                             wostrip[:, ts(k, 128)], attb[:, ts(k, 256)],
                                             start=(k == 0), stop=(k == 15))
                    ofin = sp.tile([128, RC], f32, tag="ofin")
                    nc.vector.tensor_copy(ofin[:], psf[:])
                    for mi, m in enumerate((2 * mp, 2 * mp + 1)):
                        nc.sync.dma_start(out_ext[ts(m, 128), ds(b * 256, 256)],
                                          ofin[:, ds(mi * 256, 256)])

            # ---- emission schedule ----
            for r in range(4):
                proj_chunk(r)
            for hp in range(2):
                for qs in range(4):
                    attn_block(0, hp, qs)
                    if qs in (1, 3):
                        proj_chunk(4 + 2 * hp + (qs - 1) // 2)
            do_a2a(0)
            for hp in range(2):
                for qs in range(4):
                    attn_block(1, hp, qs)
                    if hp == 0 and qs == 1:
                        oproj_load(0)
            do_a2a(1)
            oproj_ms(0, range(8))
            oproj_load(1)
            oproj_ms(1, range(8))

    nc.compile()
    return nc


def _host_prep(x, freqs_cos, freqs_sin, wq, wk, wv, wo, q_scale, k_scale):
    bfd = ml_dtypes.bfloat16
    perm = np.concatenate([np.arange(0, 64, 2), np.arange(1, 64, 2)])

    xT = np.ascontiguousarray(x.reshape(ROWS, D).T)
    x_sb = np.ascontiguousarray(
        xT.reshape(NKT, 128, ROWS).transpose(1, 0, 2).reshape(128, NKT * ROWS)
    ).astype(bfd)

    ct = np.concatenate([freqs_cos.T, freqs_cos.T], axis=1)   # [32, 4096]
    st = np.concatenate([freqs_sin.T, freqs_sin.T], axis=1)
    cos_sb = np.ascontiguousarray(np.tile(ct, (4, 1))).astype(bfd)
    sin_sb = np.ascontiguousarray(np.concatenate([-st, st, -st, st], 0)).astype(bfd)

    r = np.arange(128)[:, None]
    c = np.arange(896)[None, :]
    msk_sb = (c >= r + 384).astype(bfd)
    ind_sb = np.zeros((128, 2), bfd)
    ind_sb[0:64, 0] = 1
    ind_sb[64:128, 1] = 1
    indt_sb = np.ascontiguousarray(ind_sb.T)
    scp_sb = np.stack([np.tile(q_scale[perm], 2), np.tile(k_scale[perm], 2)],
                      axis=1).astype(np.float32)
    idn_sb = np.eye(64, dtype=bfd)
    psw_np = np.zeros((128, 128), np.float32)
    g = np.arange(128)
    psw_np[g, (g // 32 ^ 1) * 32 + g % 32] = 1.0
    psw_sb = psw_np.astype(bfd)
    sc_sb = np.stack([q_scale[perm], k_scale[perm]], axis=1).astype(np.float32)

    woT = wo.T.astype(np.float32)  # [hdim, dout]
    wo_sb = np.ascontiguousarray(
        woT.reshape(16, 128, 16, 128).transpose(1, 2, 0, 3).reshape(128, 16 * 16 * 128)
    ).astype(bfd)

    shared = dict(x_sb=x_sb, cos_sb=cos_sb, sin_sb=sin_sb, msk_sb=msk_sb,
                  ind_sb=ind_sb, indt_sb=indt_sb, scp_sb=scp_sb,
                  idn_sb=idn_sb, psw_sb=psw_sb, sc_sb=sc_sb, wo_sb=wo_sb)

    in_maps = []
    for cc in range(N_CORES):
        wq_c = wq[cc * 256:(cc + 1) * 256].reshape(4, 64, D)[:, perm].reshape(256, D)
        wqT = wq_c.T  # [D, 256]
        wq_core = np.ascontiguousarray(
            wqT.reshape(NKT, 128, 2, 128).transpose(1, 0, 2, 3).reshape(128, NKT * 256)
        ).astype(bfd)
        wk_c = wk[cc * 64:(cc + 1) * 64][perm]
        wkTd = np.concatenate([wk_c, wk_c], 0).T  # [D, 128]
        wk_core = np.ascontiguousarray(
            wkTd.reshape(NKT, 128, 128).transpose(1, 0, 2).reshape(128, NKT * 128)
        ).astype(bfd)
        wvT = wv[cc * 64:(cc + 1) * 64].T  # [D, 64]
        wv_core = np.ascontiguousarray(
            wvT.reshape(NKT, 128, 64).transpose(1, 0, 2).reshape(128, NKT * 64)
        ).astype(bfd)
        in_maps.append(dict(shared, wq_sb=wq_core, wk_sb=wk_core, wv_sb=wv_core))
    return in_maps


def kernel(x, freqs_cos, freqs_sin, wq, wk, wv, wo, q_scale, k_scale, _trace=False):
    from concourse.bass_utils import run_bass_kernel_spmd

    if "nc" not in _cache:
        _cache["nc"] = _build()
    nc = _cache["nc"]

    args = [np.asarray(a, dtype=np.float32) for a in
            (x, freqs_cos, freqs_sin, wq, wk, wv, wo, q_scale, k_scale)]
    in_maps = _host_prep(*args)
    res = run_bass_kernel_spmd(nc, in_maps, list(range(N_CORES)), trace=_trace)
    out = np.zeros((B, S, D), np.float32)
    for cc in range(N_CORES):
        oc = res.results[cc]["out"]  # [2048, 512]
        for b in range(B):
            out[b, 256 * cc:256 * (cc + 1), :] = oc[:, 256 * b:256 * (b + 1)].T
    if _trace:
        return out, res
    return out


# revision 19
# speedup vs baseline: 1.4149x; 1.1243x over previous
"""Distributed GQA attention block (dense_transformer) for 8 TRN2 NeuronCores.

Sharding: Megatron-style head sharding for QKV+attention (each core owns 4 Q
heads / 1 KV head), Ulysses-style AllToAll to switch to sequence sharding for
the output projection (each core owns 256 rows per batch).

Layouts (per core, SPMD identical graph):
  - All activations kept transposed: QT/KT [head_dim, rows] so scores come out
    as S^T [k, q] and softmax reduces over the partition axis via matmul with a
    fused ones-column in V (denominator for free).
  - RoPE handled by permuting head dims (evens|odds) in the weights on the
    host, so rotation pairs are partition halves: out = qn*cos + swap(qn)*sin±.
  - RMSNorm partition-reduction via indicator matmul; rsqrt via ACT ln/exp
    (same ACT table set as softmax exp -> zero table switches).
Compute in bf16 on the TensorEngine (f32 accumulation), f32 softmax.
"""
import sys

if '/opt/trn_rl_repo' not in sys.path:
    sys.path.insert(0, '/opt/trn_rl_repo')

import numpy as np
import ml_dtypes

N_CORES = 8
B, S, D = 2, 2048, 2048
DH = 64
HLOC = 4            # Q heads per core
ROWS = B * S        # 4096
NKT = D // 128      # 16 contraction tiles
RC = 512            # row chunk
NCHUNK = ROWS // RC # 8
EPS = 1e-6

_cache = {}


def _build():
    import concourse.mybir as mybir
    import concourse.tile as tile
    from concourse import bacc
    from concourse.bass import ts, ds

    f32 = mybir.dt.float32
    bf = mybir.dt.bfloat16
    AF = mybir.ActivationFunctionType
    MUL = mybir.AluOpType.mult

    nc = bacc.Bacc()
    x_sb = nc.declare_dram_parameter("x_sb", [128, NKT * ROWS], bf, isOutput=False)
    wq_sb = nc.declare_dram_parameter("wq_sb", [128, NKT * 2 * 128], bf, isOutput=False)
    wkv_sb = nc.declare_dram_parameter("wkv_sb", [128, NKT * 128], bf, isOutput=False)
    wo_sb = nc.declare_dram_parameter("wo_sb", [128, 16 * 16 * 128], bf, isOutput=False)
    cos_sb = nc.declare_dram_parameter("cos_sb", [128, ROWS], bf, isOutput=False)
    sin_sb = nc.declare_dram_parameter("sin_sb", [128, ROWS], bf, isOutput=False)
    msk_sb = nc.declare_dram_parameter("msk_sb", [128, 896], bf, isOutput=False)
    ind_sb = nc.declare_dram_parameter("ind_sb", [128, 2], bf, isOutput=False)
    indt_sb = nc.declare_dram_parameter("indt_sb", [2, 128], bf, isOutput=False)
    scp_sb = nc.declare_dram_parameter("scp_sb", [128, 2], f32, isOutput=False)
    idn_sb = nc.declare_dram_parameter("idn_sb", [64, 64], bf, isOutput=False)
    psw_sb = nc.declare_dram_parameter("psw_sb", [128, 128], bf, isOutput=False)
    sc_sb = nc.declare_dram_parameter("sc_sb", [64, 2], f32, isOutput=False)
    out_ext = nc.declare_dram_parameter("out", [D, 512], f32, isOutput=True)

    with tile.TileContext(nc) as tc:
        with (
            tc.tile_pool(name="cp", bufs=1) as cp,
            tc.tile_pool(name="xp", bufs=2) as xp,
            tc.tile_pool(name="wp", bufs=6) as wp,
            tc.tile_pool(name="sp", bufs=2) as sp,
            tc.tile_pool(name="ep", bufs=6) as ep,
            tc.tile_pool(name="dram", bufs=1, space="DRAM") as dram,
            tc.tile_pool(name="pmm", bufs=2, space="PSUM") as pmm,
            tc.tile_pool(name="po", bufs=2, space="PSUM") as po,
            tc.tile_pool(name="psm", bufs=2, space="PSUM") as psm,
        ):
            # ---- persistent constants ----
            wq = cp.tile([128, NKT * 2 * 128], bf)
            for i in range(4):
                nc.sync.dma_start(wq[:, ds(i * 1024, 1024)], wq_sb[:, ds(i * 1024, 1024)])
            wkvt = cp.tile([128, NKT * 128], bf)
            nc.sync.dma_start(wkvt[:], wkv_sb[:])
            msk = cp.tile([128, 896], bf)
            nc.sync.dma_start(msk[:], msk_sb[:])
            ind = cp.tile([128, 2], bf)
            nc.sync.dma_start(ind[:], ind_sb[:])
            indt = cp.tile([2, 128], bf)
            nc.sync.dma_start(indt[:], indt_sb[:])
            scp = cp.tile([128, 2], f32)
            nc.sync.dma_start(scp[:], scp_sb[:])
            idn = cp.tile([64, 64], bf)
            nc.sync.dma_start(idn[:], idn_sb[:])
            sc2 = cp.tile([64, 2], f32)
            nc.sync.dma_start(sc2[:], sc_sb[:])
            epsc = cp.tile([128, 1], f32)
            nc.gpsimd.memset(epsc[:], EPS)
            ones64 = cp.tile([1, 64], bf)
            nc.gpsimd.memset(ones64[:], 1.0)
            pswap = cp.tile([128, 128], bf)
            nc.sync.dma_start(pswap[:], psw_sb[:])

            QTn = [cp.tile([128, ROWS], bf, name=f"qtn{i}") for i in range(2)]
            KTd = [cp.tile([128, S], bf, name=f"ktd{b}") for b in range(B)]
            Vb1 = [cp.tile([128, 16 * 65], bf, name=f"vb{b}") for b in range(B)]
            attb = cp.tile([128, 16 * 256], bf)
            dsb = cp.tile([2, 16 * 256], bf)
            attb2 = cp.tile([128, 16 * 256], bf)
            dsb2 = cp.tile([2, 16 * 256], bf)

            a2a_in = [[dram.tile([1040, 256], bf, name=f"a2ain{b}{h}") for h in range(2)]
                      for b in range(B)]
            a2a_out = [[dram.tile([1040, 256], bf, name=f"a2aout{b}{h}") for h in range(2)]
                       for b in range(B)]

            # ---- norm + rope on a projection psum tile ----
            def norm_rope(ps, dst_ap, cosc, sinc):
                sq = sp.tile([128, RC], bf, tag="sq")
                nc.scalar.activation(sq[:], ps[:], AF.Square)
                ssp = psm.tile([2, RC], f32, tag="small", name="ssp")
                nc.tensor.matmul(ssp[:], ind[:], sq[:], start=True, stop=True)
                lg = sp.tile([2, RC], f32, tag="lg", bufs=4)
                nc.scalar.activation(lg[:], ssp[:], AF.Ln, scale=1.0 / 64, bias=epsc[0:2, :])
                rstd = sp.tile([2, RC], bf, tag="rstd", bufs=4)
                nc.scalar.activation(rstd[:], lg[:], AF.Exp, scale=-0.5)
                bcp = psm.tile([128, RC], f32, tag="small", name="bcpn")
                nc.tensor.matmul(bcp[:], indt[:], rstd[:], start=True, stop=True)
                bcs = sp.tile([128, RC], f32, tag="bcs")
                nc.vector.tensor_scalar(
                    out=bcs[:], in0=bcp[:], scalar1=scp[:, 0:1],
                    scalar2=None, op0=MUL)
                qn = sp.tile([128, RC], bf, tag="qn")
                nc.vector.tensor_mul(qn[:], ps[:], bcs[:])
                swps = psm.tile([128, RC], f32, tag="small", name="swps")
                nc.tensor.matmul(swps[:], pswap[:], qn[:], start=True, stop=True)
                swp = sp.tile([128, RC], bf, tag="swp")
                nc.vector.tensor_mul(swp[:], swps[:], sinc[:])
                nc.vector.tensor_mul(qn[:], qn[:], cosc[:])
                nc.vector.tensor_add(dst_ap, qn[:], swp[:])

            def norm_rope_kv(ps, dstk, b, cosc, sinc):
                sq = sp.tile([64, RC], bf, tag="sq")
                nc.scalar.activation(sq[:], ps[0:64, :], AF.Square)
                ssp = psm.tile([1, RC], f32, tag="small", name="sspk")
                nc.tensor.matmul(ssp[:], ind[0:64, 0:1], sq[:], start=True, stop=True)
                lg = sp.tile([1, RC], f32, tag="lg", bufs=4)
                nc.scalar.activation(lg[:], ssp[:], AF.Ln, scale=1.0 / 64, bias=epsc[0:1, :])
                rstd = sp.tile([1, RC], bf, tag="rstd", bufs=4)
                nc.scalar.activation(rstd[:], lg[:], AF.Exp, scale=-0.5)
                bcp = psm.tile([64, RC], f32, tag="small", name="bcpk")
                nc.tensor.matmul(bcp[:], ones64[:], rstd[:], start=True, stop=True)
                bcs = sp.tile([64, RC], f32, tag="bcs")
                nc.vector.tensor_scalar(
                    out=bcs[:], in0=bcp[:], scalar1=scp[0:64, 1:2],
                    scalar2=None, op0=MUL)
                qn = sp.tile([64, RC], bf, tag="qn")
                nc.vector.tensor_mul(qn[:], ps[0:64, :], bcs[:])
                swps = psm.tile([64, RC], f32, tag="small", name="swpsk")
                nc.tensor.matmul(swps[:], pswap[0:64, 0:64], qn[:], start=True, stop=True)
                swp = sp.tile([64, RC], bf, tag="swp")
                nc.vector.tensor_mul(swp[:], swps[:], sinc[0:64, :])
                nc.vector.tensor_mul(qn[:], qn[:], cosc[0:64, :])
                nc.vector.tensor_add(dstk[0:64, :], qn[:], swp[:])
                nc.sync.dma_start(dstk[64:128, :], dstk[0:64, :])

            # ---- one row-chunk of projections ----
            def proj_chunk(r):
                b, sl = r // 4, r % 4
                xt = xp.tile([128, NKT, RC], bf, tag="xt")
                nc.sync.dma_start(
                    xt[:], x_sb[:].rearrange("p (k q) -> p k q", k=NKT)[:, :, ds(r * RC, RC)])
                cosc = sp.tile([128, RC], bf, tag="cos")
                nc.sync.dma_start(cosc[:], cos_sb[:, ds(r * RC, RC)])
                sinc = sp.tile([128, RC], bf, tag="sin")
                nc.sync.dma_start(sinc[:], sin_sb[:, ds(r * RC, RC)])
                for hp in range(2):
                    psq = pmm.tile([128, RC], f32, tag="mm", padded_shape=[128, 1024], name=f"psq{hp}")
                    for k in range(NKT):
                        nc.tensor.matmul(psq[:], wq[:, ds((k * 2 + hp) * 128, 128)],
                                         xt[:, k, :], start=(k == 0), stop=(k == NKT - 1))
                    norm_rope(psq, QTn[hp][:, ds(r * RC, RC)], cosc, sinc)
                pskv = pmm.tile([128, RC], f32, tag="mm", padded_shape=[128, 1024])
                for k in range(NKT):
                    nc.tensor.matmul(pskv[:], wkvt[:, ts(k, 128)], xt[:, k, :],
                                     start=(k == 0), stop=(k == NKT - 1))
                norm_rope_kv(pskv, KTd[b][:, ds(sl * RC, RC)], b, cosc, sinc)
                vtmp = sp.tile([64, RC], bf, tag="vtmp")
                nc.vector.tensor_copy(vtmp[:], pskv[64:128, :])
                for t4 in range(4):
                    tp = psm.tile([128, 64], bf, tag="small", name="tp")
                    nc.tensor.transpose(tp[:], vtmp[:, ts(t4, 128)], idn[:])
                    gt = sl * 4 + t4
                    nc.vector.tensor_copy(Vb1[b][:, ds(gt * 65, 64)], tp[:])
                    nc.vector.memset(Vb1[b][:, ds(gt * 65 + 64, 1)], 1.0)

            # ---- one attention block: batch b, head-pair hp, q-slice qs ----
            def attn_block(b, hp, qs):
                psO = [po.tile([65, RC], f32, tag="o", name=f"psO{t}") for t in range(2)]
                jmax = qs * 4 + 3

                def scores(j):
                    dj = j - qs * 4
                    p = dj * 128 if dj >= 0 else 0
                    N = RC - p
                    qb = b * S + qs * RC + p
                    psS = pmm.tile([128, 2 * RC], f32, tag="mm", name="psS")
                    nc.tensor.matmul(psS[:, 0:N], KTd[b][0:64, ts(j, 128)],
                                     QTn[hp][0:64, ds(qb, N)], start=True, stop=True,
                                     tile_position=(0, 0))
                    nc.tensor.matmul(psS[:, RC:RC + N], KTd[b][64:128, ts(j, 128)],
                                     QTn[hp][64:128, ds(qb, N)], start=True, stop=True,
                                     tile_position=(64, 0))
                    E = ep.tile([128, 2 * RC], bf, tag="E")
                    e3 = E[:].rearrange("p (t q) -> p t q", t=2)
                    s3 = psS[:].rearrange("p (t q) -> p t q", t=2)
                    nc.scalar.activation(e3[:, :, p:RC], s3[:, :, 0:N], AF.Exp, scale=0.125)
                    if dj >= 0:
                        nc.vector.tensor_mul(
                            e3[:, :, ds(p, 128)], e3[:, :, ds(p, 128)],
                            msk[:, ds(384, 128)].unsqueeze(1).broadcast_to([128, 2, 128]))
                    return p, E

                def av(j, pE):
                    p, E = pE
                    for t in range(2):
                        nc.tensor.matmul(psO[t][:, p:RC], Vb1[b][:, ds(j * 65, 65)],
                                         E[:, ds(t * RC + p, RC - p)],
                                         start=(j == 0), stop=(j == jmax))

                pend = {0: scores(0)}
                if jmax >= 1:
                    pend[1] = scores(1)
                for j in range(2, jmax + 1):
                    pend[j] = scores(j)
                    av(j - 2, pend.pop(j - 2))
                for j in sorted(pend):
                    av(j, pend[j])
                for t in range(2):
                    hl = 2 * hp + t
                    on65 = sp.tile([65, RC], bf, tag="on")
                    nc.vector.tensor_copy(on65[:], psO[t][:])
                    nc.sync.dma_start(
                        a2a_in[b][hp][ds(130 * (2 * qs) + 65 * t, 65), :], on65[:, 0:256])
                    nc.sync.dma_start(
                        a2a_in[b][hp][ds(130 * (2 * qs + 1) + 65 * t, 65), :],
                        on65[:, 256:512])

            def do_a2a(b, h):
                nc.gpsimd.collective_compute(
                    "AllToAll", mybir.AluOpType.bypass,
                    replica_groups=[list(range(N_CORES))],
                    ins=[a2a_in[b][h].opt()], outs=[a2a_out[b][h].opt()])

            def oproj_load(b, h):
                ab = attb if b == 0 else attb2
                db = dsb if b == 0 else dsb2
                av3 = a2a_out[b][h][:].rearrange("(j t e) n -> e j t n", j=8, t=2)
                ab3 = ab[:].rearrange("(t dh) (j hpj n) -> dh j hpj t n",
                                      t=2, dh=64, j=8, hpj=2)
                for t in range(2):
                    nc.sync.dma_start(ab3[:, :, h, t, :], av3[0:64, :, t, :])
                nc.sync.dma_start(
                    db[:].rearrange("t (j hpj n) -> t j hpj n", j=8, hpj=2)[:, :, h, :],
                    av3[64, :, :, :].transpose([1, 0, 2]))

            def norm_attb(b, h):
                ab = attb if b == 0 else attb2
                db = dsb if b == 0 else dsb2
                rd = sp.tile([2, 8 * 256], bf, tag="rd")
                with nc.allow_low_precision(reason="softmax denom recip bf16"):
                    nc.vector.reciprocal(
                        rd[:].rearrange("t (j n) -> t j n", j=8),
                        db[:].rearrange("t (j hpj n) -> t j hpj n", j=8, hpj=2)[:, :, h, :])
                for i, k in enumerate(range(h, 16, 2)):
                    bcd = psm.tile([128, 256], f32, tag="small", name="bcd")
                    nc.tensor.matmul(bcd[:], indt[:], rd[:, ts(i, 256)],
                                     start=True, stop=True)
                    nc.vector.tensor_mul(ab[:, ts(k, 256)], ab[:, ts(k, 256)], bcd[:])

            def oproj_ms(b, mpairs):
                ab = attb if b == 0 else attb2
                for mp in mpairs:
                    psf = pmm.tile([128, RC], f32, tag="mm", padded_shape=[128, 1024], name="psf")
                    for mi, m in enumerate((2 * mp, 2 * mp + 1)):
                        wostrip = wp.tile([128, 2048], bf, tag="wo")
                        nc.sync.dma_start(wostrip[:], wo_sb[:, ds(m * 2048, 2048)])
                        for k in range(16):
                            nc.tensor.matmul(psf[:, ds(mi * 256, 256)],
                                             wostrip[:, ts(k, 128)], ab[:, ts(k, 256)],
                                             start=(k == 0), stop=(k == 15))
                    ofin = sp.tile([128, RC], f32, tag="ofin")
                    nc.vector.tensor_copy(ofin[:], psf[:])
                    for mi, m in enumerate((2 * mp, 2 * mp + 1)):
                        nc.sync.dma_start(out_ext[ts(m, 128), ds(b * 256, 256)],
                                          ofin[:, ds(mi * 256, 256)])

            # ---- emission schedule ----
            for r in range(4):
                proj_chunk(r)
            for hp in range(2):
                for qs in range(4):
                    attn_block(0, hp, qs)
                    if qs in (1, 3):
                        proj_chunk(4 + 2 * hp + (qs - 1) // 2)
                do_a2a(0, hp)
            for qs in range(4):
                attn_block(1, 0, qs)
                if qs == 0:
                    oproj_load(0, 0)
                    norm_attb(0, 0)
                elif qs == 1:
                    oproj_load(0, 1)
                    norm_attb(0, 1)
                elif qs == 2:
                    oproj_ms(0, [0])
                else:
                    oproj_ms(0, [1])
            do_a2a(1, 0)
            for qs in range(4):
                attn_block(1, 1, qs)
                if qs == 0:
                    oproj_ms(0, [2, 3])
                elif qs == 1:
                    oproj_ms(0, [4, 5])
                elif qs == 2:
                    oproj_load(1, 0)
                    norm_attb(1, 0)
            do_a2a(1, 1)
            oproj_ms(0, [6, 7])
            oproj_load(1, 1)
            norm_attb(1, 1)
            oproj_ms(1, range(8))

    nc.compile()
    return nc


def _host_prep(x, freqs_cos, freqs_sin, wq, wk, wv, wo, q_scale, k_scale):
    bfd = ml_dtypes.bfloat16
    perm = np.concatenate([np.arange(0, 64, 2), np.arange(1, 64, 2)])

    xT = np.ascontiguousarray(x.reshape(ROWS, D).T)
    x_sb = np.ascontiguousarray(
        xT.reshape(NKT, 128, ROWS).transpose(1, 0, 2).reshape(128, NKT * ROWS)
    ).astype(bfd)

    ct = np.concatenate([freqs_cos.T, freqs_cos.T], axis=1)   # [32, 4096]
    st = np.concatenate([freqs_sin.T, freqs_sin.T], axis=1)
    cos_sb = np.ascontiguousarray(np.tile(ct, (4, 1))).astype(bfd)
    sin_sb = np.ascontiguousarray(np.concatenate([-st, st, -st, st], 0)).astype(bfd)

    r = np.arange(128)[:, None]
    c = np.arange(896)[None, :]
    msk_sb = (c >= r + 384).astype(bfd)
    ind_sb = np.zeros((128, 2), bfd)
    ind_sb[0:64, 0] = 1
    ind_sb[64:128, 1] = 1
    indt_sb = np.ascontiguousarray(ind_sb.T)
    scp_sb = np.stack([np.tile(q_scale[perm], 2), np.tile(k_scale[perm], 2)],
                      axis=1).astype(np.float32)
    idn_sb = np.eye(64, dtype=bfd)
    psw_np = np.zeros((128, 128), np.float32)
    g = np.arange(128)
    psw_np[g, (g // 32 ^ 1) * 32 + g % 32] = 1.0
    psw_sb = psw_np.astype(bfd)
    sc_sb = np.stack([q_scale[perm], k_scale[perm]], axis=1).astype(np.float32)

    woT = wo.T.astype(np.float32)  # [hdim, dout]
    wo_sb = np.ascontiguousarray(
        woT.reshape(16, 128, 16, 128).transpose(1, 2, 0, 3).reshape(128, 16 * 16 * 128)
    ).astype(bfd)

    shared = dict(x_sb=x_sb, cos_sb=cos_sb, sin_sb=sin_sb, msk_sb=msk_sb,
                  ind_sb=ind_sb, indt_sb=indt_sb, scp_sb=scp_sb,
                  idn_sb=idn_sb, psw_sb=psw_sb, sc_sb=sc_sb, wo_sb=wo_sb)

    in_maps = []
    for cc in range(N_CORES):
        wq_c = wq[cc * 256:(cc + 1) * 256].reshape(4, 64, D)[:, perm].reshape(256, D)
        wqT = wq_c.T  # [D, 256]
        wq_core = np.ascontiguousarray(
            wqT.reshape(NKT, 128, 2, 128).transpose(1, 0, 2, 3).reshape(128, NKT * 256)
        ).astype(bfd)
        wk_c = wk[cc * 64:(cc + 1) * 64][perm]
        wv_c = wv[cc * 64:(cc + 1) * 64]
        wkvT = np.concatenate([wk_c, wv_c], 0).T  # [D, 128]: cols 0:64=K(perm), 64:128=V
        wkv_core = np.ascontiguousarray(
            wkvT.reshape(NKT, 128, 128).transpose(1, 0, 2).reshape(128, NKT * 128)
        ).astype(bfd)
        in_maps.append(dict(shared, wq_sb=wq_core, wkv_sb=wkv_core))
    return in_maps


def kernel(x, freqs_cos, freqs_sin, wq, wk, wv, wo, q_scale, k_scale, _trace=False):
    from concourse.bass_utils import run_bass_kernel_spmd

    if "nc" not in _cache:
        _cache["nc"] = _build()
    nc = _cache["nc"]

    args = [np.asarray(a, dtype=np.float32) for a in
            (x, freqs_cos, freqs_sin, wq, wk, wv, wo, q_scale, k_scale)]
    in_maps = _host_prep(*args)
    res = run_bass_kernel_spmd(nc, in_maps, list(range(N_CORES)), trace=_trace)
    out = np.zeros((B, S, D), np.float32)
    for cc in range(N_CORES):
        oc = res.results[cc]["out"]  # [2048, 512]
        for b in range(B):
            out[b, 256 * cc:256 * (cc + 1), :] = oc[:, 256 * b:256 * (b + 1)].T
    if _trace:
        return out, res
    return out


# revision 21
# speedup vs baseline: 1.4613x; 1.0328x over previous
"""Distributed GQA attention block (dense_transformer) for 8 TRN2 NeuronCores.

Sharding: Megatron-style head sharding for QKV+attention (each core owns 4 Q
heads / 1 KV head), Ulysses-style AllToAll to switch to sequence sharding for
the output projection (each core owns 256 rows per batch).

Layouts (per core, SPMD identical graph):
  - All activations kept transposed: QT/KT [head_dim, rows] so scores come out
    as S^T [k, q] and softmax reduces over the partition axis via matmul with a
    fused ones-column in V (denominator for free).
  - RoPE handled by permuting head dims (evens|odds) in the weights on the
    host, so rotation pairs are partition halves: out = qn*cos + swap(qn)*sin±.
  - RMSNorm partition-reduction via indicator matmul; rsqrt via ACT ln/exp
    (same ACT table set as softmax exp -> zero table switches).
Compute in bf16 on the TensorEngine (f32 accumulation), f32 softmax.
"""
import sys

if '/opt/trn_rl_repo' not in sys.path:
    sys.path.insert(0, '/opt/trn_rl_repo')

import numpy as np
import ml_dtypes

N_CORES = 8
B, S, D = 2, 2048, 2048
DH = 64
HLOC = 4            # Q heads per core
ROWS = B * S        # 4096
NKT = D // 128      # 16 contraction tiles
RC = 512            # row chunk
NCHUNK = ROWS // RC # 8
EPS = 1e-6

_cache = {}


def _build():
    import concourse.mybir as mybir
    import concourse.tile as tile
    from concourse import bacc
    from concourse.bass import ts, ds

    f32 = mybir.dt.float32
    bf = mybir.dt.bfloat16
    AF = mybir.ActivationFunctionType
    MUL = mybir.AluOpType.mult

    nc = bacc.Bacc()
    x_sb = nc.declare_dram_parameter("x_sb", [128, NKT * ROWS], bf, isOutput=False)
    wq_sb = nc.declare_dram_parameter("wq_sb", [128, NKT * 2 * 128], bf, isOutput=False)
    wkv_sb = nc.declare_dram_parameter("wkv_sb", [128, NKT * 128], bf, isOutput=False)
    wo_sb = nc.declare_dram_parameter("wo_sb", [128, 16 * 16 * 128], bf, isOutput=False)
    cos_sb = nc.declare_dram_parameter("cos_sb", [128, ROWS], bf, isOutput=False)
    sin_sb = nc.declare_dram_parameter("sin_sb", [128, ROWS], bf, isOutput=False)
    msk_sb = nc.declare_dram_parameter("msk_sb", [128, 896], bf, isOutput=False)
    ind_sb = nc.declare_dram_parameter("ind_sb", [128, 2], bf, isOutput=False)
    indt_sb = nc.declare_dram_parameter("indt_sb", [2, 128], bf, isOutput=False)
    scp_sb = nc.declare_dram_parameter("scp_sb", [128, 2], f32, isOutput=False)
    idn_sb = nc.declare_dram_parameter("idn_sb", [64, 64], bf, isOutput=False)
    psw_sb = nc.declare_dram_parameter("psw_sb", [128, 128], bf, isOutput=False)
    sc_sb = nc.declare_dram_parameter("sc_sb", [64, 2], f32, isOutput=False)
    out_ext = nc.declare_dram_parameter("out", [D, 512], f32, isOutput=True)

    with tile.TileContext(nc) as tc:
        with (
            tc.tile_pool(name="cp", bufs=1) as cp,
            tc.tile_pool(name="xp", bufs=3) as xp,
            tc.tile_pool(name="wp", bufs=6) as wp,
            tc.tile_pool(name="sp", bufs=2) as sp,
            tc.tile_pool(name="ep", bufs=6) as ep,
            tc.tile_pool(name="dram", bufs=1, space="DRAM") as dram,
            tc.tile_pool(name="pmm", bufs=2, space="PSUM") as pmm,
            tc.tile_pool(name="po", bufs=2, space="PSUM") as po,
            tc.tile_pool(name="psm", bufs=2, space="PSUM") as psm,
        ):
            # ---- persistent constants ----
            wq = cp.tile([128, NKT * 2 * 128], bf)
            for i in range(4):
                nc.sync.dma_start(wq[:, ds(i * 1024, 1024)], wq_sb[:, ds(i * 1024, 1024)])
            wkvt = cp.tile([128, NKT * 128], bf)
            nc.sync.dma_start(wkvt[:], wkv_sb[:])
            msk = cp.tile([128, 896], bf)
            nc.sync.dma_start(msk[:], msk_sb[:])
            ind = cp.tile([128, 2], bf)
            nc.sync.dma_start(ind[:], ind_sb[:])
            indt = cp.tile([2, 128], bf)
            nc.sync.dma_start(indt[:], indt_sb[:])
            scp = cp.tile([128, 2], f32)
            nc.sync.dma_start(scp[:], scp_sb[:])
            idn = cp.tile([64, 64], bf)
            nc.sync.dma_start(idn[:], idn_sb[:])
            sc2 = cp.tile([64, 2], f32)
            nc.sync.dma_start(sc2[:], sc_sb[:])
            epsc = cp.tile([128, 1], f32)
            nc.gpsimd.memset(epsc[:], EPS)
            ones64 = cp.tile([1, 64], bf)
            nc.gpsimd.memset(ones64[:], 1.0)
            pswap = cp.tile([128, 128], bf)
            nc.sync.dma_start(pswap[:], psw_sb[:])

            QTn = [cp.tile([128, ROWS], bf, name=f"qtn{i}") for i in range(2)]
            KTd = [cp.tile([128, S], bf, name=f"ktd{b}") for b in range(B)]
            Vb1 = [cp.tile([128, 16 * 65], bf, name=f"vb{b}") for b in range(B)]
            attb = cp.tile([128, 16 * 256], bf)
            dsb = cp.tile([2, 16 * 256], bf)
            attb2 = cp.tile([128, 16 * 256], bf)
            dsb2 = cp.tile([2, 16 * 256], bf)

            a2a_in = [[dram.tile([1040, 256], bf, name=f"a2ain{b}{h}") for h in range(2)]
                      for b in range(B)]
            a2a_out = [[dram.tile([1040, 256], bf, name=f"a2aout{b}{h}") for h in range(2)]
                       for b in range(B)]

            # ---- norm + rope on a projection psum tile ----
            def norm_rope(ps, dst_ap, cosc, sinc):
                sq = sp.tile([128, RC], bf, tag="sq")
                nc.scalar.activation(sq[:], ps[:], AF.Square)
                ssp = psm.tile([2, RC], f32, tag="small", name="ssp")
                nc.tensor.matmul(ssp[:], ind[:], sq[:], start=True, stop=True)
                lg = sp.tile([2, RC], f32, tag="lg", bufs=4)
                nc.scalar.activation(lg[:], ssp[:], AF.Ln, scale=1.0 / 64, bias=epsc[0:2, :])
                rstd = sp.tile([2, RC], bf, tag="rstd", bufs=4)
                nc.scalar.activation(rstd[:], lg[:], AF.Exp, scale=-0.5)
                bcp = psm.tile([128, RC], f32, tag="small", name="bcpn")
                nc.tensor.matmul(bcp[:], indt[:], rstd[:], start=True, stop=True)
                bcs = sp.tile([128, RC], f32, tag="bcs")
                nc.vector.tensor_scalar(
                    out=bcs[:], in0=bcp[:], scalar1=scp[:, 0:1],
                    scalar2=None, op0=MUL)
                qn = sp.tile([128, RC], bf, tag="qn")
                nc.vector.tensor_mul(qn[:], ps[:], bcs[:])
                swps = psm.tile([128, RC], f32, tag="small", name="swps")
                nc.tensor.matmul(swps[:], pswap[:], qn[:], start=True, stop=True)
                swp = sp.tile([128, RC], bf, tag="swp")
                nc.vector.tensor_mul(swp[:], swps[:], sinc[:])
                nc.vector.tensor_mul(qn[:], qn[:], cosc[:])
                nc.vector.tensor_add(dst_ap, qn[:], swp[:])

            def norm_rope_kv(ps, dstk, b, cosc, sinc):
                sq = sp.tile([64, RC], bf, tag="sq")
                nc.scalar.activation(sq[:], ps[0:64, :], AF.Square)
                ssp = psm.tile([1, RC], f32, tag="small", name="sspk")
                nc.tensor.matmul(ssp[:], ind[0:64, 0:1], sq[:], start=True, stop=True)
                lg = sp.tile([1, RC], f32, tag="lg", bufs=4)
                nc.scalar.activation(lg[:], ssp[:], AF.Ln, scale=1.0 / 64, bias=epsc[0:1, :])
                rstd = sp.tile([1, RC], bf, tag="rstd", bufs=4)
                nc.scalar.activation(rstd[:], lg[:], AF.Exp, scale=-0.5)
                bcp = psm.tile([64, RC], f32, tag="small", name="bcpk")
                nc.tensor.matmul(bcp[:], ones64[:], rstd[:], start=True, stop=True)
                bcs = sp.tile([64, RC], f32, tag="bcs")
                nc.vector.tensor_scalar(
                    out=bcs[:], in0=bcp[:], scalar1=scp[0:64, 1:2],
                    scalar2=None, op0=MUL)
                qn = sp.tile([64, RC], bf, tag="qn")
                nc.vector.tensor_mul(qn[:], ps[0:64, :], bcs[:])
                swps = psm.tile([64, RC], f32, tag="small", name="swpsk")
                nc.tensor.matmul(swps[:], pswap[0:64, 0:64], qn[:], start=True, stop=True)
                swp = sp.tile([64, RC], bf, tag="swp")
                nc.vector.tensor_mul(swp[:], swps[:], sinc[0:64, :])
                nc.vector.tensor_mul(qn[:], qn[:], cosc[0:64, :])
                nc.vector.tensor_add(dstk[0:64, :], qn[:], swp[:])
                nc.sync.dma_start(dstk[64:128, :], dstk[0:64, :])

            # ---- one row-chunk of projections ----
            def proj_chunk(r):
                b, sl = r // 4, r % 4
                xt = xp.tile([128, NKT, RC], bf, tag="xt")
                nc.sync.dma_start(
                    xt[:], x_sb[:].rearrange("p (k q) -> p k q", k=NKT)[:, :, ds(r * RC, RC)])
                cosc = sp.tile([128, RC], bf, tag="cos")
                nc.sync.dma_start(cosc[:], cos_sb[:, ds(r * RC, RC)])
                sinc = sp.tile([128, RC], bf, tag="sin")
                nc.sync.dma_start(sinc[:], sin_sb[:, ds(r * RC, RC)])
                for hp in range(2):
                    psq = pmm.tile([128, RC], f32, tag="mm", padded_shape=[128, 1024], name=f"psq{hp}")
                    for k in range(NKT):
                        nc.tensor.matmul(psq[:], wq[:, ds((k * 2 + hp) * 128, 128)],
                                         xt[:, k, :], start=(k == 0), stop=(k == NKT - 1))
                    norm_rope(psq, QTn[hp][:, ds(r * RC, RC)], cosc, sinc)
                pskv = pmm.tile([128, RC], f32, tag="mm", padded_shape=[128, 1024])
                for k in range(NKT):
                    nc.tensor.matmul(pskv[:], wkvt[:, ts(k, 128)], xt[:, k, :],
                                     start=(k == 0), stop=(k == NKT - 1))
                norm_rope_kv(pskv, KTd[b][:, ds(sl * RC, RC)], b, cosc, sinc)
                vtmp = sp.tile([64, RC], bf, tag="vtmp")
                nc.vector.tensor_copy(vtmp[:], pskv[64:128, :])
                for t4 in range(4):
                    tp = psm.tile([128, 64], bf, tag="small", name="tp")
                    nc.tensor.transpose(tp[:], vtmp[:, ts(t4, 128)], idn[:])
                    gt = sl * 4 + t4
                    nc.vector.tensor_copy(Vb1[b][:, ds(gt * 65, 64)], tp[:])
                    nc.vector.memset(Vb1[b][:, ds(gt * 65 + 64, 1)], 1.0)

            # ---- one attention block: batch b, head-pair hp, q-slice qs ----
            def attn_block(b, hp, qs):
                psO = [po.tile([65, RC], f32, tag="o", name=f"psO{t}") for t in range(2)]
                jmax = qs * 4 + 3

                def scores(j):
                    dj = j - qs * 4
                    p = dj * 128 if dj >= 0 else 0
                    N = RC - p
                    qb = b * S + qs * RC + p
                    psS = pmm.tile([128, 2 * RC], f32, tag="mm", name="psS")
                    nc.tensor.matmul(psS[:, 0:N], KTd[b][0:64, ts(j, 128)],
                                     QTn[hp][0:64, ds(qb, N)], start=True, stop=True,
                                     tile_position=(0, 0))
                    nc.tensor.matmul(psS[:, RC:RC + N], KTd[b][64:128, ts(j, 128)],
                                     QTn[hp][64:128, ds(qb, N)], start=True, stop=True,
                                     tile_position=(64, 0))
                    E = ep.tile([128, 2 * RC], bf, tag="E")
                    e3 = E[:].rearrange("p (t q) -> p t q", t=2)
                    s3 = psS[:].rearrange("p (t q) -> p t q", t=2)
                    nc.scalar.activation(e3[:, :, p:RC], s3[:, :, 0:N], AF.Exp, scale=0.125)
                    if dj >= 0:
                        nc.vector.tensor_mul(
                            e3[:, :, ds(p, 128)], e3[:, :, ds(p, 128)],
                            msk[:, ds(384, 128)].unsqueeze(1).broadcast_to([128, 2, 128]))
                    return p, E

                def av(j, pE):
                    p, E = pE
                    for t in range(2):
                        nc.tensor.matmul(psO[t][:, p:RC], Vb1[b][:, ds(j * 65, 65)],
                                         E[:, ds(t * RC + p, RC - p)],
                                         start=(j == 0), stop=(j == jmax))

                pend = {0: scores(0)}
                if jmax >= 1:
                    pend[1] = scores(1)
                for j in range(2, jmax + 1):
                    pend[j] = scores(j)
                    av(j - 2, pend.pop(j - 2))
                for j in sorted(pend):
                    av(j, pend[j])
                for t in range(2):
                    hl = 2 * hp + t
                    on65 = sp.tile([65, RC], bf, tag="on")
                    nc.vector.tensor_copy(on65[:], psO[t][:])
                    nc.sync.dma_start(
                        a2a_in[b][hp][ds(130 * (2 * qs) + 65 * t, 65), :], on65[:, 0:256])
                    nc.sync.dma_start(
                        a2a_in[b][hp][ds(130 * (2 * qs + 1) + 65 * t, 65), :],
                        on65[:, 256:512])

            def do_a2a(b, h):
                nc.gpsimd.collective_compute(
                    "AllToAll", mybir.AluOpType.bypass,
                    replica_groups=[list(range(N_CORES))],
                    ins=[a2a_in[b][h].opt()], outs=[a2a_out[b][h].opt()])

            def oproj_load(b, h):
                ab = attb if b == 0 else attb2
                db = dsb if b == 0 else dsb2
                av3 = a2a_out[b][h][:].rearrange("(j t e) n -> e j t n", j=8, t=2)
                ab3 = ab[:].rearrange("(t dh) (j hpj n) -> dh j hpj t n",
                                      t=2, dh=64, j=8, hpj=2)
                for t in range(2):
                    nc.sync.dma_start(ab3[:, :, h, t, :], av3[0:64, :, t, :])
                nc.sync.dma_start(
                    db[:].rearrange("t (j hpj n) -> t j hpj n", j=8, hpj=2)[:, :, h, :],
                    av3[64, :, :, :].transpose([1, 0, 2]))

            def norm_attb(b, h):
                ab = attb if b == 0 else attb2
                db = dsb if b == 0 else dsb2
                rd = sp.tile([2, 8 * 256], bf, tag="rd")
                with nc.allow_low_precision(reason="softmax denom recip bf16"):
                    nc.vector.reciprocal(
                        rd[:].rearrange("t (j n) -> t j n", j=8),
                        db[:].rearrange("t (j hpj n) -> t j hpj n", j=8, hpj=2)[:, :, h, :])
                for i, k in enumerate(range(h, 16, 2)):
                    bcd = psm.tile([128, 256], f32, tag="small", name="bcd")
                    nc.tensor.matmul(bcd[:], indt[:], rd[:, ts(i, 256)],
                                     start=True, stop=True)
                    nc.vector.tensor_mul(ab[:, ts(k, 256)], ab[:, ts(k, 256)], bcd[:])

            def oproj_ms(b, mpairs):
                ab = attb if b == 0 else attb2
                for mp in mpairs:
                    psf = pmm.tile([128, RC], f32, tag="mm", padded_shape=[128, 1024], name="psf")
                    for mi, m in enumerate((2 * mp, 2 * mp + 1)):
                        wostrip = wp.tile([128, 2048], bf, tag="wo")
                        nc.sync.dma_start(wostrip[:], wo_sb[:, ds(m * 2048, 2048)])
                        for k in range(16):
                            nc.tensor.matmul(psf[:, ds(mi * 256, 256)],
                                             wostrip[:, ts(k, 128)], ab[:, ts(k, 256)],
                                             start=(k == 0), stop=(k == 15))
                    ofin = sp.tile([128, RC], f32, tag="ofin")
                    nc.vector.tensor_copy(ofin[:], psf[:])
                    for mi, m in enumerate((2 * mp, 2 * mp + 1)):
                        nc.sync.dma_start(out_ext[ts(m, 128), ds(b * 256, 256)],
                                          ofin[:, ds(mi * 256, 256)])

            # ---- emission schedule ----
            for r in range(4):
                proj_chunk(r)
            for hp in range(2):
                for qs in range(4):
                    attn_block(0, hp, qs)
                    if qs in (1, 3):
                        proj_chunk(4 + 2 * hp + (qs - 1) // 2)
                do_a2a(0, hp)
            for qs in range(4):
                attn_block(1, 0, qs)
                if qs == 1:
                    oproj_load(0, 0)
                    norm_attb(0, 0)
                elif qs == 2:
                    oproj_load(0, 1)
                    norm_attb(0, 1)
                elif qs == 3:
                    oproj_ms(0, [0, 1])
            do_a2a(1, 0)
            for qs in range(4):
                attn_block(1, 1, qs)
                if qs == 0:
                    oproj_ms(0, [2, 3])
                elif qs == 1:
                    oproj_ms(0, [4, 5])
                elif qs == 2:
                    oproj_load(1, 0)
                    norm_attb(1, 0)
            do_a2a(1, 1)
            oproj_ms(0, [6, 7])
            oproj_load(1, 1)
            norm_attb(1, 1)
            oproj_ms(1, range(8))

    nc.compile()
    return nc


def _host_prep(x, freqs_cos, freqs_sin, wq, wk, wv, wo, q_scale, k_scale):
    bfd = ml_dtypes.bfloat16
    perm = np.concatenate([np.arange(0, 64, 2), np.arange(1, 64, 2)])

    xT = np.ascontiguousarray(x.reshape(ROWS, D).T)
    x_sb = np.ascontiguousarray(
        xT.reshape(NKT, 128, ROWS).transpose(1, 0, 2).reshape(128, NKT * ROWS)
    ).astype(bfd)

    ct = np.concatenate([freqs_cos.T, freqs_cos.T], axis=1)   # [32, 4096]
    st = np.concatenate([freqs_sin.T, freqs_sin.T], axis=1)
    cos_sb = np.ascontiguousarray(np.tile(ct, (4, 1))).astype(bfd)
    sin_sb = np.ascontiguousarray(np.concatenate([-st, st, -st, st], 0)).astype(bfd)

    r = np.arange(128)[:, None]
    c = np.arange(896)[None, :]
    msk_sb = (c >= r + 384).astype(bfd)
    ind_sb = np.zeros((128, 2), bfd)
    ind_sb[0:64, 0] = 1
    ind_sb[64:128, 1] = 1
    indt_sb = np.ascontiguousarray(ind_sb.T)
    scp_sb = np.stack([np.tile(q_scale[perm], 2), np.tile(k_scale[perm], 2)],
                      axis=1).astype(np.float32)
    idn_sb = np.eye(64, dtype=bfd)
    psw_np = np.zeros((128, 128), np.float32)
    g = np.arange(128)
    psw_np[g, (g // 32 ^ 1) * 32 + g % 32] = 1.0
    psw_sb = psw_np.astype(bfd)
    sc_sb = np.stack([q_scale[perm], k_scale[perm]], axis=1).astype(np.float32)

    woT = wo.T.astype(np.float32)  # [hdim, dout]
    wo_sb = np.ascontiguousarray(
        woT.reshape(16, 128, 16, 128).transpose(1, 2, 0, 3).reshape(128, 16 * 16 * 128)
    ).astype(bfd)

    shared = dict(x_sb=x_sb, cos_sb=cos_sb, sin_sb=sin_sb, msk_sb=msk_sb,
                  ind_sb=ind_sb, indt_sb=indt_sb, scp_sb=scp_sb,
                  idn_sb=idn_sb, psw_sb=psw_sb, sc_sb=sc_sb, wo_sb=wo_sb)

    in_maps = []
    for cc in range(N_CORES):
        wq_c = wq[cc * 256:(cc + 1) * 256].reshape(4, 64, D)[:, perm].reshape(256, D)
        wqT = wq_c.T  # [D, 256]
        wq_core = np.ascontiguousarray(
            wqT.reshape(NKT, 128, 2, 128).transpose(1, 0, 2, 3).reshape(128, NKT * 256)
        ).astype(bfd)
        wk_c = wk[cc * 64:(cc + 1) * 64][perm]
        wv_c = wv[cc * 64:(cc + 1) * 64]
        wkvT = np.concatenate([wk_c, wv_c], 0).T  # [D, 128]: cols 0:64=K(perm), 64:128=V
        wkv_core = np.ascontiguousarray(
            wkvT.reshape(NKT, 128, 128).transpose(1, 0, 2).reshape(128, NKT * 128)
        ).astype(bfd)
        in_maps.append(dict(shared, wq_sb=wq_core, wkv_sb=wkv_core))
    return in_maps


def kernel(x, freqs_cos, freqs_sin, wq, wk, wv, wo, q_scale, k_scale, _trace=False):
    from concourse.bass_utils import run_bass_kernel_spmd

    if "nc" not in _cache:
        _cache["nc"] = _build()
    nc = _cache["nc"]

    args = [np.asarray(a, dtype=np.float32) for a in
            (x, freqs_cos, freqs_sin, wq, wk, wv, wo, q_scale, k_scale)]
    in_maps = _host_prep(*args)
    res = run_bass_kernel_spmd(nc, in_maps, list(range(N_CORES)), trace=_trace)
    out = np.zeros((B, S, D), np.float32)
    for cc in range(N_CORES):
        oc = res.results[cc]["out"]  # [2048, 512]
        for b in range(B):
            out[b, 256 * cc:256 * (cc + 1), :] = oc[:, 256 * b:256 * (b + 1)].T
    if _trace:
        return out, res
    return out
